# revision 13
# baseline (speedup 1.0000x reference)
"""Trainium2 kernel for nn_ConnectedThresholdLayer (gated connected-filter on
morphological max-trees + pixel reconstruction).

Mathematical reformulation (exactly equivalent to the reference on valid
trees, which setup_inputs always produces):

  The reference computes, per (b,c) tree, S[n] = sum of s[k] over the
  root->n path (pointer-doubling with K=12 covers depth < 4096; actual
  random-recursive-tree depth is ~35), with
      s[k] = gate[k] * (level[k] - level[parent[k]]),  s[root] = level[root]
      gate[k] = (sigmoid(a_scaled - thr_norm) >= 0.5)  ==  (attr[k] >= thr)
  (min-max scaling is strictly monotone, so the 0.5-sigmoid threshold
  reduces exactly to the raw comparison), then out[pix] = S[node[pix]].

  Path sums over a tree are an Euler-tour prefix scan: entering node k adds
  s[k], leaving subtracts it; the running sum at k's entry event equals
  S[k].  Leaf exit events are elided (the stream shrinks 2N -> ~1.5N): a
  leaf's entry slot carries its attr with the SIGN BIT flipped, so the scan
  gate (attr >= thr) reads 0 there (no carry pollution) while a second gate
  (attr <= -thr) recovers the leaf's own contribution in a post-scan add:

      out[j] = inclusive_scan(w2)[j] + (cross_partition_carry + root_level)
               + (attr[j] <= -thr) * w1[j]
      w2[j]  = (attr[j] >= thr) * w1[j],   w1[j] = lv[j] - plv[j]

  The host derives the (data-independent) tour layout from the int32
  `parent` tensor alone; the device does all floating-point math: gates,
  residues, the ~393k-element prefix scan per tree (per-partition scan +
  PE-matmul cross-partition carry), fully dense — no data-dependent
  addressing on device.

Precision: level payloads travel as fp16 (entry/exit contribution pairs are
exact fp16 negations — swapped operands — so path-sum error grows only with
tree depth ~35, not stream length).  attr stays fp32: the gate compare must
not flip near the threshold.  The scan state is fp32 in hardware regardless
of operand dtype; only the stored output rounds to fp16.

Sharding: trees are independent per (b,c); the 24 trees go 3-per-NeuronCore
across 8 cores (data parallel, zero cross-device communication).  Per tree,
the attr stream, level stream, and result stream ride separate DMA queues
(SP / Activation / PE engines) so transfers overlap.

Host does ONLY integer index planning (from `parent` / `pixel_to_node`) and
data marshaling (reordering input copies into event order, sign-bit flips
on the uint32 view, inverse map on the returned scan); every floating-point
operation on attr/level/thr values runs on the NeuronCores.
"""

import numpy as np

P = 128            # SBUF partitions
TREES_PER_CORE = 3
N_CORES = 8

_CACHE = {}


# ----------------------------------------------------------------------------
# Host-side integer planning (uses only `parent` / `pixel_to_node`)
# ----------------------------------------------------------------------------

def _tree_plan(parent):
    """parent: (N,) int with parent[n] < n for n >= 1.

    Returns ev_enter (N,) int64: position of each node's entry event in the
    2N-long Euler event stream.  Root (node 0) is excluded from the stream;
    positions 0 and 2N-1 are zero-contribution pads, and ev_enter[0] = 0
    (the running sum there is 0; the root's base level is added globally).
    """
    N = parent.shape[0]
    par = parent.astype(np.int64)
    ar = np.arange(N)

    # depth (= #edges to root) via pointer doubling with absorbing root
    val = (ar != 0).astype(np.int64)
    a = par.copy()
    a[0] = 0
    for _ in range(20):
        if not a.any():
            break
        val = val + val[a]
        a = a[a]
    depth = val
    maxd = int(depth.max())
    if maxd >= 4096:
        return None, None, maxd

    # subtree sizes, bottom-up by depth level
    size = np.ones(N, np.int64)
    order = np.argsort(depth, kind="stable")
    bounds = np.searchsorted(depth[order], np.arange(maxd + 2))
    for d in range(maxd, 0, -1):
        nodes = order[bounds[d]:bounds[d + 1]]
        if len(nodes) == 0:
            continue
        size += np.bincount(par[nodes], weights=size[nodes],
                            minlength=N).astype(np.int64)

    # prefix of earlier-sibling subtree sizes (children visited in index order)
    sibord = np.argsort(par[1:], kind="stable") + 1
    sz = size[sibord]
    cs = np.cumsum(sz) - sz
    pgroup = par[sibord]
    first = np.ones(len(sibord), bool)
    first[1:] = pgroup[1:] != pgroup[:-1]
    base = np.where(first, cs, 0)
    np.maximum.accumulate(base, out=base)
    bss = np.zeros(N, np.int64)
    bss[sibord] = cs - base

    # preorder index = path-sum of (1 + bss) excluding root, via doubling
    c = 1 + bss
    c[0] = 0
    S = c
    a = par.copy()
    a[0] = 0
    for _ in range(20):
        if not a.any():
            break
        S = S + S[a]
        a = a[a]
    pre = S
    ev_enter = 2 * pre - depth
    ev_enter[0] = 0
    return ev_enter, size, maxd


def _host_preprocess(attr, level, thr, parent, pixel_to_node):
    """Returns (in_maps for 8 cores, q (T, HW) int32 slot positions, F)."""
    B, C, N = attr.shape
    T = B * C
    twoN = 2 * N
    attr2 = np.ascontiguousarray(attr.reshape(T, N))
    level2 = np.ascontiguousarray(level.reshape(T, N))
    par2 = np.ascontiguousarray(parent.reshape(T, N))
    pix2 = pixel_to_node.reshape(T, -1)

    # pass 1: plan all trees, find the common padded slot count
    plans = []
    maxM = 0
    nr = np.arange(1, N)
    for t in range(T):
        ev_enter, size, maxd = _tree_plan(par2[t])
        if maxd >= 4096:
            # reference's K=12 pointer doubling truncates paths longer than
            # 4096; the Euler scan computes the untruncated sum -> not
            # equivalent. Caller must use the exact fallback.
            return None, None, None
        ev_exit = ev_enter + 2 * size - 1
        pr = par2[t]
        nch = np.bincount(pr[1:], minlength=N)
        leaf = nch == 0
        keep = np.ones(twoN, bool)
        keep[ev_exit[leaf]] = False    # drop leaf exits
        keep[twoN - 1] = False         # drop trailing root pad
        newpos = (np.cumsum(keep) - 1).astype(np.int64)
        M = int(newpos[-1] + 1)
        maxM = max(maxM, M)
        plans.append((ev_enter, ev_exit, leaf, newpos))
    F = -(-maxM // (8 * P)) * 8        # slots per partition, padded to 8

    MP = P * F
    evA = np.zeros((T, MP), np.float32)
    evL = np.zeros((T, 2 * MP), np.float16)    # per row: [lv | plv]
    q = np.empty((T, pix2.shape[1]), np.int32)
    for t in range(T):
        ev_enter, ev_exit, leaf, newpos = plans[t]
        at, lv, pr = attr2[t], level2[t], par2[t]
        en2 = newpos[ev_enter]
        ex2 = newpos[ev_exit]
        plv = lv[pr[nr]]
        ni = nr[~leaf[1:]]             # internal non-root nodes
        nl = nr[leaf[1:]]              # leaf nodes
        evA[t, en2[ni]] = at[ni]
        evA[t, ex2[ni]] = at[ni]
        afl = at[nl].copy()
        afl.view(np.uint32)[:] ^= 0x80000000   # sign-bit flip (integer op)
        evA[t, en2[nl]] = afl
        el = evL[t, :MP]
        ep = evL[t, MP:]
        el[en2[nr]] = lv[nr]
        ep[en2[nr]] = plv
        el[ex2[ni]] = plv[~leaf[1:]]   # swapped operands => exact negation
        ep[ex2[ni]] = lv[ni]
        q[t] = en2[np.clip(pix2[t], 0, N - 1)].astype(np.int32)

    thr_f = np.float32(thr.reshape(-1)[0])
    negthr = np.array(thr_f, np.float32)
    negthr.view(np.uint32)[...] ^= 0x80000000   # sign-bit flip (integer op)
    tri = np.triu(np.ones((P, P), np.float32), 1)   # tri[k,m]=1 iff k<m
    in_maps = []
    for c in range(N_CORES):
        tt = slice(c * TREES_PER_CORE, (c + 1) * TREES_PER_CORE)
        params = np.zeros((P, 4 * TREES_PER_CORE), np.float32)
        for k in range(TREES_PER_CORE):
            params[:, 4 * k] = thr_f
            params[:, 4 * k + 1] = negthr
            params[:, 4 * k + 2] = level2[c * TREES_PER_CORE + k, 0]
        in_maps.append({
            "evA": evA[tt].reshape(TREES_PER_CORE * P, F),
            "evL": evL[tt].reshape(TREES_PER_CORE, 2, P, F)
                .transpose(0, 2, 1, 3).reshape(TREES_PER_CORE * P, 2 * F),
            "params": params, "tri": tri})
    return in_maps, q, F


# ----------------------------------------------------------------------------
# Device program
# ----------------------------------------------------------------------------

def _build_nc(F, repeat=1, cfg=None):
    import concourse.bacc as bacc
    import concourse.mybir as mybir
    import concourse.tile as tile
    import concourse.bass as bass

    cfg = cfg or {}
    bufs = cfg.get("bufs", 2)

    f32 = mybir.dt.float32
    f16 = mybir.dt.float16
    op = mybir.AluOpType
    TP = TREES_PER_CORE * P

    nc = bacc.Bacc("TRN2", target_bir_lowering=False, debug=False,
                   num_devices=N_CORES)
    evA = nc.dram_tensor("evA", [TP, F], f32, kind="ExternalInput")
    evL = nc.dram_tensor("evL", [TP, 2 * F], f16, kind="ExternalInput")
    params = nc.dram_tensor("params", [P, 4 * TREES_PER_CORE], f32,
                            kind="ExternalInput")
    triT = nc.dram_tensor("tri", [P, P], f32, kind="ExternalInput")
    Rout = nc.dram_tensor("R", [TP, F], f16, kind="ExternalOutput")

    with tile.TileContext(nc) as tc:
        with tc.tile_pool(name="sbuf", bufs=bufs) as pool, \
             tc.tile_pool(name="psum", space=bass.MemorySpace.PSUM,
                          bufs=2) as ppool:
            zero16 = pool.tile([P, 1], f16, tag="z16", bufs=1)
            nc.vector.memset(zero16[:], 0.0)
            prm = pool.tile([P, 4 * TREES_PER_CORE], f32, tag="prm", bufs=1)
            nc.sync.dma_start(prm, params.ap()[:, :])
            tri = pool.tile([P, P], f32, tag="tri", bufs=1)
            nc.sync.dma_start(tri, triT.ap()[:, :])
            for i, t in enumerate([tt % TREES_PER_CORE for tt in
                                   range(TREES_PER_CORE * repeat)]):
                rows = slice(t * P, (t + 1) * P)
                ea = pool.tile([P, F], f32, tag="ea")
                nc.sync.dma_start(ea, evA.ap()[rows, :])
                el = pool.tile([P, 2 * F], f16, tag="el")
                nc.scalar.dma_start(el, evL.ap()[rows, :])

                # w1 = level - parent_level (exact fp16 negation pairs)
                w1 = pool.tile([P, F], f16, tag="w1")
                nc.gpsimd.tensor_tensor(out=w1[:], in0=el[:, 0:F],
                                        in1=el[:, F:2 * F], op=op.subtract)
                # w2 = (attr >= thr) * w1 with fused per-partition row sums
                w2 = pool.tile([P, F], f16, tag="w2")
                rowsum = pool.tile([P, 1], f32, tag="rowsum")
                nc.vector.scalar_tensor_tensor(
                    out=w2[:], in0=ea[:], scalar=prm[:, 4 * t:4 * t + 1],
                    in1=w1[:], op0=op.is_ge, op1=op.mult,
                    accum_out=rowsum[:])
                # o1 = (attr <= -thr) * w1: recovers leaf contributions
                # (their attr rides with a flipped sign bit)
                o1 = pool.tile([P, F], f16, tag="o1")
                nc.vector.scalar_tensor_tensor(
                    out=o1[:], in0=ea[:],
                    scalar=prm[:, 4 * t + 1:4 * t + 2],
                    in1=w1[:], op0=op.is_le, op1=op.mult)

                # cross-partition exclusive prefix of rowsums on the (idle)
                # PE: excl[p] = sum_{k<p} rowsum[k] via strict-upper ones
                excl = ppool.tile([P, 1], f32, tag="excl")
                nc.tensor.matmul(excl[:], tri[:], rowsum[:])
                carry2 = pool.tile([P, 1], f32, tag="carry2")
                nc.vector.tensor_tensor(out=carry2[:], in0=excl[:],
                                        in1=prm[:, 4 * t + 2:4 * t + 3],
                                        op=op.add)

                # rf = zero-seeded prefix scan of w2 (fp32 state, fp16 out);
                # the carry joins in the final add, off the critical path
                rf = pool.tile([P, F], f16, tag="rf")
                nc.vector.tensor_tensor_scan(
                    out=rf[:], data0=w2[:],
                    data1=zero16[:].to_broadcast([P, F]),
                    initial=0.0, op0=op.add, op1=op.add)

                # out = (rf + carry2) + o1: per-partition bias add on the
                # Activation engine, then tensor add on Pool
                rfc = pool.tile([P, F], f16, tag="rfc")
                nc.scalar.add(rfc[:], rf[:], carry2[:, 0:1])
                ot = pool.tile([P, F], f16, tag="ot")
                nc.gpsimd.tensor_tensor(out=ot[:], in0=rfc[:], in1=o1[:],
                                        op=op.add)
                nc.gpsimd.dma_start(Rout.ap()[rows, :], ot[:])
    nc.compile()
    return nc


def _get_nc(F):
    key = ("nc", F)
    if key not in _CACHE:
        _CACHE[key] = _build_nc(F)
    return _CACHE[key]


# ----------------------------------------------------------------------------
# Fallback: exact f32 emulation of the reference (invalid/cyclic trees only)
# ----------------------------------------------------------------------------

def _fallback_reference(attr, level, thr, parent, pixel_to_node):
    B, C, N = attr.shape
    # replicate reference's scaled-sigmoid gate semantics
    amin = attr.min(-1, keepdims=True)
    amax = attr.max(-1, keepdims=True)
    denom = np.maximum(amax - amin, np.float32(1e-6))
    a_s = ((attr - amin) / denom).astype(np.float32)
    t_n = ((np.float32(thr.reshape(-1)[0]) - amin) / denom).astype(np.float32)
    d = (a_s - t_n).astype(np.float32)
    soft = (1.0 / (1.0 + np.exp(-d.astype(np.float64)))).astype(np.float32)
    gate = (soft >= 0.5).astype(np.float32)
    pixel_to_node = np.clip(pixel_to_node, 0, N - 1)
    pl = np.take_along_axis(level, np.clip(parent, 0, N - 1).astype(np.int64),
                            axis=-1)
    s = gate * (level - pl)
    s[..., 0] = level[..., 0]
    s = np.concatenate([s, np.zeros((B, C, 1), np.float32)], axis=-1)
    p = np.concatenate([np.clip(parent, 0, N).astype(np.int32),
                        np.full((B, C, 1), N, np.int32)], axis=-1)
    p[..., 0] = N
    S = s.astype(np.float32)
    pp = p.astype(np.int64)
    for _ in range(12):
        S = (S + np.take_along_axis(S, pp, axis=-1)).astype(np.float32)
        pp = np.take_along_axis(pp, pp, axis=-1)
    S = S[..., :N]
    out = np.take_along_axis(S, pixel_to_node.astype(np.int64), axis=-1)
    HW = pixel_to_node.shape[-1]
    H = int(np.sqrt(HW))
    return out.reshape(B, C, H, HW // H).astype(np.float32)


# ----------------------------------------------------------------------------
# Entry point
# ----------------------------------------------------------------------------

def kernel(attr, level, thr_raw, parent, pixel_to_node):
    attr = np.asarray(attr, np.float32)
    level = np.asarray(level, np.float32)
    thr_raw = np.asarray(thr_raw, np.float32)
    parent = np.asarray(parent)
    pixel_to_node = np.asarray(pixel_to_node)
    B, C, N = attr.shape
    HW = pixel_to_node.shape[-1]
    H = int(np.sqrt(HW))

    par2 = parent.reshape(-1, N)
    valid = bool(np.all(par2[:, 1:] < np.arange(1, N)) and np.all(par2 >= 0)
                 and float(thr_raw.reshape(-1)[0]) > 0.0)
    if not valid or B * C != N_CORES * TREES_PER_CORE:
        return _fallback_reference(attr, level, thr_raw, parent, pixel_to_node)

    in_maps, q, F = _host_preprocess(attr, level, thr_raw, parent,
                                     pixel_to_node)
    if in_maps is None:  # depth >= 4096: doubling truncation applies
        return _fallback_reference(attr, level, thr_raw, parent,
                                   pixel_to_node)
    try:
        nc = _get_nc(F)
        from concourse.bass_utils import run_bass_kernel_spmd
        res = run_bass_kernel_spmd(nc, in_maps, core_ids=list(range(N_CORES)))
    except Exception as e:  # infra failure: still return a correct result
        import traceback
        traceback.print_exc()
        print(f"kernel: device path failed ({type(e).__name__}); "
              "falling back to host emulation")
        return _fallback_reference(attr, level, thr_raw, parent,
                                   pixel_to_node)

    out = np.empty((B * C, HW), np.float32)
    for c in range(N_CORES):
        R = res.results[c]["R"].astype(np.float32).reshape(TREES_PER_CORE,
                                                           P * F)
        for k in range(TREES_PER_CORE):
            t = c * TREES_PER_CORE + k
            out[t] = R[k][q[t]]
    return out.reshape(B, C, H, HW // H)


# revision 24
# speedup vs baseline: 2.2962x; 2.2962x over previous
"""Trainium2 kernel for nn_ConnectedThresholdLayer (gated connected-filter on
morphological max-trees + pixel reconstruction).

Mathematical reformulation (exactly equivalent to the reference on valid
trees, which setup_inputs always produces):

  The reference computes, per (b,c) tree, S[n] = sum of s[k] over the
  root->n path (pointer-doubling with K=12 covers depth < 4096; actual
  random-recursive-tree depth is ~35), with
      s[k] = gate[k] * (level[k] - level[parent[k]]),  s[root] = level[root]
      gate[k] = (sigmoid(a_scaled - thr_norm) >= 0.5)  ==  (attr[k] >= thr)
  (min-max scaling is strictly monotone, so the 0.5-sigmoid threshold
  reduces exactly to the raw comparison), then out[pix] = S[node[pix]].

  Path sums over a tree are an Euler-tour prefix scan: entering node k adds
  s[k], leaving subtracts it; the running sum at k's entry event equals
  S[k].  Leaf exit events are elided (the stream shrinks 2N -> ~1.5N): a
  leaf's entry slot carries its attr with the SIGN BIT flipped, so the scan
  gate (attr >= thr) reads 0 there (no carry pollution) while a second gate
  (attr <= -thr) recovers the leaf's own contribution in a post-scan add:

      out[j] = inclusive_scan(w2)[j] + (cross_partition_carry + root_level)
               + (attr[j] <= -thr) * w1[j]
      w2[j]  = (attr[j] >= thr) * w1[j],   w1[j] = lv[j] - plv[j]

  The host derives the (data-independent) tour layout from the int32
  `parent` tensor alone; the device does all floating-point math: gates,
  residues, the ~393k-element prefix scan per tree (per-partition scan +
  PE-matmul cross-partition carry), fully dense — no data-dependent
  addressing on device.

Precision: level payloads travel as fp16 (entry/exit contribution pairs are
exact fp16 negations — swapped operands — so path-sum error grows only with
tree depth ~35, not stream length).  attr stays fp32: the gate compare must
not flip near the threshold.  The scan state is fp32 in hardware regardless
of operand dtype; only the stored output rounds to fp16.

Sharding: trees are independent per (b,c); the 24 trees go 3-per-NeuronCore
across 8 cores (data parallel, zero cross-device communication).  Per tree,
the attr stream, level stream, and result stream ride separate DMA queues
(SP / Activation / PE engines) so transfers overlap.

Host does ONLY integer index planning (from `parent` / `pixel_to_node`) and
data marshaling (reordering input copies into event order, sign-bit flips
on the uint32 view, inverse map on the returned scan); every floating-point
operation on attr/level/thr values runs on the NeuronCores.
"""

import numpy as np

P = 128            # SBUF partitions
TREES_PER_CORE = 3
N_CORES = 8
LEVEL_DTYPE = np.float16  # u8 halves DMA but the DVE u8 path is slower on HW

_CACHE = {}


# ----------------------------------------------------------------------------
# Host-side integer planning (uses only `parent` / `pixel_to_node`)
# ----------------------------------------------------------------------------

def _tree_plan(parent):
    """parent: (N,) int with parent[n] < n for n >= 1.

    Returns ev_enter (N,) int64: position of each node's entry event in the
    2N-long Euler event stream.  Root (node 0) is excluded from the stream;
    positions 0 and 2N-1 are zero-contribution pads, and ev_enter[0] = 0
    (the running sum there is 0; the root's base level is added globally).
    """
    N = parent.shape[0]
    par = parent.astype(np.int64)
    ar = np.arange(N)

    # depth (= #edges to root) via pointer doubling with absorbing root
    val = (ar != 0).astype(np.int64)
    a = par.copy()
    a[0] = 0
    for _ in range(20):
        if not a.any():
            break
        val = val + val[a]
        a = a[a]
    depth = val
    maxd = int(depth.max())
    if maxd >= 4096:
        return None, None, maxd

    # subtree sizes, bottom-up by depth level
    size = np.ones(N, np.int64)
    order = np.argsort(depth, kind="stable")
    bounds = np.searchsorted(depth[order], np.arange(maxd + 2))
    for d in range(maxd, 0, -1):
        nodes = order[bounds[d]:bounds[d + 1]]
        if len(nodes) == 0:
            continue
        size += np.bincount(par[nodes], weights=size[nodes],
                            minlength=N).astype(np.int64)

    # prefix of earlier-sibling subtree sizes (children visited in index order)
    sibord = np.argsort(par[1:], kind="stable") + 1
    sz = size[sibord]
    cs = np.cumsum(sz) - sz
    pgroup = par[sibord]
    first = np.ones(len(sibord), bool)
    first[1:] = pgroup[1:] != pgroup[:-1]
    base = np.where(first, cs, 0)
    np.maximum.accumulate(base, out=base)
    bss = np.zeros(N, np.int64)
    bss[sibord] = cs - base

    # preorder index = path-sum of (1 + bss) excluding root, via doubling
    c = 1 + bss
    c[0] = 0
    S = c
    a = par.copy()
    a[0] = 0
    for _ in range(20):
        if not a.any():
            break
        S = S + S[a]
        a = a[a]
    pre = S
    ev_enter = 2 * pre - depth
    ev_enter[0] = 0
    return ev_enter, size, maxd


def _host_preprocess(attr, level, thr, parent, pixel_to_node):
    """Returns (in_maps for 8 cores, q (T, HW) int32 slot positions, F)."""
    B, C, N = attr.shape
    T = B * C
    twoN = 2 * N
    attr2 = np.ascontiguousarray(attr.reshape(T, N))
    level2 = np.ascontiguousarray(level.reshape(T, N))
    par2 = np.ascontiguousarray(parent.reshape(T, N))
    pix2 = pixel_to_node.reshape(T, -1)

    # pass 1: plan all trees, find the common padded slot count
    plans = []
    maxM = 0
    nr = np.arange(1, N)
    for t in range(T):
        ev_enter, size, maxd = _tree_plan(par2[t])
        if maxd >= 4096:
            # reference's K=12 pointer doubling truncates paths longer than
            # 4096; the Euler scan computes the untruncated sum -> not
            # equivalent. Caller must use the exact fallback.
            return None, None, None
        ev_exit = ev_enter + 2 * size - 1
        pr = par2[t]
        nch = np.bincount(pr[1:], minlength=N)
        leaf = nch == 0
        keep = np.ones(twoN, bool)
        keep[ev_exit[leaf]] = False    # drop leaf exits
        keep[twoN - 1] = False         # drop trailing root pad
        newpos = (np.cumsum(keep) - 1).astype(np.int64)
        M = int(newpos[-1] + 1)
        maxM = max(maxM, M)
        plans.append((ev_enter, ev_exit, leaf, newpos))
    F = -(-maxM // (8 * P)) * 8        # slots per partition, padded to 8

    MP = P * F
    evA = np.zeros((T, MP), np.float32)
    evL = np.zeros((T, 2 * MP), LEVEL_DTYPE)   # per row: [lv | plv]
    q = np.empty((T, pix2.shape[1]), np.int32)
    for t in range(T):
        ev_enter, ev_exit, leaf, newpos = plans[t]
        at, lv, pr = attr2[t], level2[t], par2[t]
        en2 = newpos[ev_enter]
        ex2 = newpos[ev_exit]
        plv = lv[pr[nr]]
        ni = nr[~leaf[1:]]             # internal non-root nodes
        nl = nr[leaf[1:]]              # leaf nodes
        evA[t, en2[ni]] = at[ni]
        evA[t, ex2[ni]] = at[ni]
        afl = at[nl].copy()
        afl.view(np.uint32)[:] ^= 0x80000000   # sign-bit flip (integer op)
        evA[t, en2[nl]] = afl
        el = evL[t, :MP]
        ep = evL[t, MP:]
        el[en2[nr]] = lv[nr]
        ep[en2[nr]] = plv
        el[ex2[ni]] = plv[~leaf[1:]]   # swapped operands => exact negation
        ep[ex2[ni]] = lv[ni]
        q[t] = en2[np.clip(pix2[t], 0, N - 1)].astype(np.int32)

    thr_f = np.float32(thr.reshape(-1)[0])
    negthr = np.array(thr_f, np.float32)
    negthr.view(np.uint32)[...] ^= 0x80000000   # sign-bit flip (integer op)
    tri = np.triu(np.ones((P, P), np.float32), 1)   # tri[k,m]=1 iff k<m
    in_maps = []
    for c in range(N_CORES):
        tt = slice(c * TREES_PER_CORE, (c + 1) * TREES_PER_CORE)
        params = np.zeros((P, 4 * TREES_PER_CORE), np.float32)
        for k in range(TREES_PER_CORE):
            params[:, 4 * k] = thr_f
            params[:, 4 * k + 1] = negthr
            params[:, 4 * k + 2] = level2[c * TREES_PER_CORE + k, 0]
        in_maps.append({
            "evA": evA[tt].reshape(TREES_PER_CORE * P, F),
            "evL": evL[tt].reshape(TREES_PER_CORE, 2, P, F)
                .transpose(0, 2, 1, 3).reshape(TREES_PER_CORE * P, 2 * F),
            "params": params, "tri": tri})
    return in_maps, q, F


# ----------------------------------------------------------------------------
# Device program
# ----------------------------------------------------------------------------

def _build_nc(F, repeat=1, cfg=None):
    import concourse.bacc as bacc
    import concourse.mybir as mybir
    import concourse.tile as tile
    import concourse.bass as bass

    cfg = cfg or {}
    bufs = cfg.get("bufs", 2)
    version = cfg.get("version", "v3")   # "v3" | "v4"
    w1_eng = cfg.get("w1_eng", "vector")
    gneg_eng = cfg.get("gneg_eng", "act")  # v4: "act" | "dve"
    scan_bypass = cfg.get("scan_bypass", True)
    o1_via_act = cfg.get("o1_via_act", True)
    noscan = cfg.get("noscan", False)
    odma = cfg.get("odma", "scalar")     # "scalar" | "gpsimd" | "sync"
    mode = cfg.get("mode", "full")       # "full" | "dmaonly"

    f32 = mybir.dt.float32
    f16 = mybir.dt.float16
    op = mybir.AluOpType
    TP = TREES_PER_CORE * P

    nc = bacc.Bacc("TRN2", target_bir_lowering=False, debug=False,
                   num_devices=N_CORES)
    evA = nc.dram_tensor("evA", [TP, F], f32, kind="ExternalInput")
    lvdt = {np.dtype(np.uint8): mybir.dt.uint8,
            np.dtype(np.float16): f16}[np.dtype(LEVEL_DTYPE)]
    evL = nc.dram_tensor("evL", [TP, 2 * F], lvdt, kind="ExternalInput")
    params = nc.dram_tensor("params", [P, 4 * TREES_PER_CORE], f32,
                            kind="ExternalInput")
    triT = nc.dram_tensor("tri", [P, P], f32, kind="ExternalInput")
    Rout = nc.dram_tensor("R", [TP, F], f16, kind="ExternalOutput")

    with tile.TileContext(nc) as tc:
        with tc.tile_pool(name="sbuf", bufs=bufs) as pool, \
             tc.tile_pool(name="psum", space=bass.MemorySpace.PSUM,
                          bufs=2) as ppool:
            zero16 = pool.tile([P, 1], f16, tag="z16", bufs=1)
            nc.vector.memset(zero16[:], 0.0)
            prm = pool.tile([P, 4 * TREES_PER_CORE], f32, tag="prm", bufs=1)
            nc.sync.dma_start(prm, params.ap()[:, :])
            tri = pool.tile([P, P], f32, tag="tri", bufs=1)
            nc.sync.dma_start(tri, triT.ap()[:, :])
            odma_eng = {"scalar": nc.scalar, "gpsimd": nc.gpsimd,
                        "sync": nc.sync}[odma]
            for i, t in enumerate([tt % TREES_PER_CORE for tt in
                                   range(TREES_PER_CORE * repeat)]):
                rows = slice(t * P, (t + 1) * P)
                ea = pool.tile([P, F], f32, tag="ea")
                nc.sync.dma_start(ea, evA.ap()[rows, :])
                el = pool.tile([P, 2 * F], lvdt, tag="el")
                nc.sync.dma_start(el, evL.ap()[rows, :])
                if mode == "dmaonly":
                    odma_eng.dma_start(Rout.ap()[rows, :],
                                       el[:, 0:F])
                    continue

                # w1 = level - parent_level (exact fp16 negation pairs)
                w1_e = {"gpsimd": nc.gpsimd, "vector": nc.vector}[w1_eng]
                w1 = pool.tile([P, F], f16, tag="w1")
                w1_e.tensor_tensor(out=w1[:], in0=el[:, 0:F],
                                   in1=el[:, F:2 * F], op=op.subtract)
                # w2 = (attr >= thr) * w1 with fused per-partition row sums
                w2 = pool.tile([P, F], f16, tag="w2")
                rowsum = pool.tile([P, 1], f32, tag="rowsum")
                nc.vector.scalar_tensor_tensor(
                    out=w2[:], in0=ea[:], scalar=prm[:, 4 * t:4 * t + 1],
                    in1=w1[:], op0=op.is_ge, op1=op.mult,
                    accum_out=rowsum[:])

                # cross-partition exclusive prefix of rowsums on the (idle)
                # PE: excl[p] = sum_{k<p} rowsum[k] via strict-upper ones
                excl = ppool.tile([P, 1], f32, tag="excl")
                nc.tensor.matmul(excl[:], tri[:], rowsum[:])
                carry2 = pool.tile([P, 1], f32, tag="carry2")
                nc.vector.tensor_tensor(out=carry2[:], in0=excl[:],
                                        in1=prm[:, 4 * t + 2:4 * t + 3],
                                        op=op.add)

                ot = pool.tile([P, F], f16, tag="ot")
                if version == "v3":
                    # o1 = (attr <= -thr)*w1; carry-seeded scan; final
                    # 3-operand add on DVE
                    o1 = pool.tile([P, F], f16, tag="o1")
                    if o1_via_act:
                        sg = pool.tile([P, F], f16, tag="sg")
                        nc.scalar.activation(
                            sg[:], ea[:], mybir.ActivationFunctionType.Sign,
                            bias=prm[:, 4 * t + 1:4 * t + 2], scale=-1.0)
                        gneg = pool.tile([P, F], f16, tag="gneg")
                        nc.scalar.activation(
                            gneg[:], sg[:], mybir.ActivationFunctionType.Relu)
                        nc.vector.tensor_tensor(out=o1[:], in0=gneg[:],
                                                in1=w1[:], op=op.mult)
                    else:
                        nc.vector.scalar_tensor_tensor(
                            out=o1[:], in0=ea[:],
                            scalar=prm[:, 4 * t + 1:4 * t + 2],
                            in1=w1[:], op0=op.is_le, op1=op.mult)
                    rf = pool.tile([P, F], f16, tag="rf")
                    if noscan:
                        nc.vector.tensor_tensor(out=rf[:], in0=w2[:],
                                                in1=o1[:], op=op.add)
                    elif scan_bypass:
                        nc.vector.tensor_tensor_scan(
                            out=rf[:], data0=w2[:], data1=w2[:],
                            initial=0.0, op0=op.add, op1=op.bypass)
                    else:
                        nc.vector.tensor_tensor_scan(
                            out=rf[:], data0=w2[:],
                            data1=zero16[:].to_broadcast([P, F]),
                            initial=0.0, op0=op.add, op1=op.add)
                    nc.vector.scalar_tensor_tensor(
                        out=ot[:], in0=rf[:], scalar=carry2[:, 0:1],
                        in1=o1[:], op0=op.add, op1=op.add)
                else:
                    # gneg = (attr <= -thr) as 0/1 (leaf slots carry
                    # sign-flipped attr, so Sign(-attr-thr) is +1 there)
                    gneg = pool.tile([P, F], f16, tag="gneg")
                    if gneg_eng == "act":
                        sg = pool.tile([P, F], f16, tag="sg")
                        nc.scalar.activation(
                            sg[:], ea[:], mybir.ActivationFunctionType.Sign,
                            bias=prm[:, 4 * t + 1:4 * t + 2], scale=-1.0)
                        nc.scalar.activation(
                            gneg[:], sg[:], mybir.ActivationFunctionType.Relu)
                    else:
                        nc.vector.tensor_scalar(
                            out=gneg[:], in0=ea[:],
                            scalar1=prm[:, 4 * t + 1:4 * t + 2],
                            scalar2=None, op0=op.is_le)
                    # o1 = gneg * w1: the leaf contributions
                    o1 = pool.tile([P, F], f16, tag="o1")
                    nc.gpsimd.tensor_tensor(out=o1[:], in0=gneg[:],
                                            in1=w1[:], op=op.mult)
                    # d0 = w2 - shift1(o1): with data1=o1 below, the scan
                    # emits inclscan(w2)[j] + o1[j] directly (telescoping).
                    # f32: the fp16-fp16 difference must stay exact or the
                    # telescoping drifts over the 393k-slot stream
                    d0 = pool.tile([P, F], f32, tag="d0")
                    nc.gpsimd.tensor_copy(d0[:, 0:1], w2[:, 0:1])
                    nc.gpsimd.tensor_tensor(out=d0[:, 1:F], in0=w2[:, 1:F],
                                            in1=o1[:, 0:F - 1],
                                            op=op.subtract)
                    # the scan IS the final output: fp32 state, fp16 out
                    if noscan:
                        nc.vector.tensor_tensor(out=ot[:], in0=w2[:],
                                                in1=o1[:], op=op.add)
                    else:
                        nc.vector.tensor_tensor_scan(
                            out=ot[:], data0=d0[:], data1=o1[:],
                            initial=carry2[:, 0:1], op0=op.add, op1=op.add)
                odma_eng.dma_start(Rout.ap()[rows, :], ot[:])
    nc.compile()
    return nc


def _get_nc(F):
    key = ("nc", F)
    if key not in _CACHE:
        _CACHE[key] = _build_nc(F)
    return _CACHE[key]


# ----------------------------------------------------------------------------
# Fallback: exact f32 emulation of the reference (invalid/cyclic trees only)
# ----------------------------------------------------------------------------

def _fallback_reference(attr, level, thr, parent, pixel_to_node):
    B, C, N = attr.shape
    # replicate reference's scaled-sigmoid gate semantics
    amin = attr.min(-1, keepdims=True)
    amax = attr.max(-1, keepdims=True)
    denom = np.maximum(amax - amin, np.float32(1e-6))
    a_s = ((attr - amin) / denom).astype(np.float32)
    t_n = ((np.float32(thr.reshape(-1)[0]) - amin) / denom).astype(np.float32)
    d = (a_s - t_n).astype(np.float32)
    soft = (1.0 / (1.0 + np.exp(-d.astype(np.float64)))).astype(np.float32)
    gate = (soft >= 0.5).astype(np.float32)
    pixel_to_node = np.clip(pixel_to_node, 0, N - 1)
    pl = np.take_along_axis(level, np.clip(parent, 0, N - 1).astype(np.int64),
                            axis=-1)
    s = gate * (level - pl)
    s[..., 0] = level[..., 0]
    s = np.concatenate([s, np.zeros((B, C, 1), np.float32)], axis=-1)
    p = np.concatenate([np.clip(parent, 0, N).astype(np.int32),
                        np.full((B, C, 1), N, np.int32)], axis=-1)
    p[..., 0] = N
    S = s.astype(np.float32)
    pp = p.astype(np.int64)
    for _ in range(12):
        S = (S + np.take_along_axis(S, pp, axis=-1)).astype(np.float32)
        pp = np.take_along_axis(pp, pp, axis=-1)
    S = S[..., :N]
    out = np.take_along_axis(S, pixel_to_node.astype(np.int64), axis=-1)
    HW = pixel_to_node.shape[-1]
    H = int(np.sqrt(HW))
    return out.reshape(B, C, H, HW // H).astype(np.float32)


# ----------------------------------------------------------------------------
# Entry point
# ----------------------------------------------------------------------------

def kernel(attr, level, thr_raw, parent, pixel_to_node):
    attr = np.asarray(attr, np.float32)
    level = np.asarray(level, np.float32)
    thr_raw = np.asarray(thr_raw, np.float32)
    parent = np.asarray(parent)
    pixel_to_node = np.asarray(pixel_to_node)
    B, C, N = attr.shape
    HW = pixel_to_node.shape[-1]
    H = int(np.sqrt(HW))

    par2 = parent.reshape(-1, N)
    valid = bool(np.all(par2[:, 1:] < np.arange(1, N)) and np.all(par2 >= 0)
                 and float(thr_raw.reshape(-1)[0]) > 0.0)
    if not valid or B * C != N_CORES * TREES_PER_CORE:
        return _fallback_reference(attr, level, thr_raw, parent, pixel_to_node)

    in_maps, q, F = _host_preprocess(attr, level, thr_raw, parent,
                                     pixel_to_node)
    if in_maps is None:  # depth >= 4096: doubling truncation applies
        return _fallback_reference(attr, level, thr_raw, parent,
                                   pixel_to_node)
    try:
        nc = _get_nc(F)
        from concourse.bass_utils import run_bass_kernel_spmd
        res = run_bass_kernel_spmd(nc, in_maps, core_ids=list(range(N_CORES)))
    except Exception as e:  # infra failure: still return a correct result
        import traceback
        traceback.print_exc()
        print(f"kernel: device path failed ({type(e).__name__}); "
              "falling back to host emulation")
        return _fallback_reference(attr, level, thr_raw, parent,
                                   pixel_to_node)

    out = np.empty((B * C, HW), np.float32)
    for c in range(N_CORES):
        R = res.results[c]["R"].astype(np.float32).reshape(TREES_PER_CORE,
                                                           P * F)
        for k in range(TREES_PER_CORE):
            t = c * TREES_PER_CORE + k
            out[t] = R[k][q[t]]
    return out.reshape(B, C, H, HW // H)


# revision 30
# speedup vs baseline: 2.4477x; 1.0660x over previous
"""Trainium2 kernel for nn_ConnectedThresholdLayer (gated connected-filter on
morphological max-trees + pixel reconstruction).

Mathematical reformulation (exactly equivalent to the reference on valid
trees, which setup_inputs always produces):

  The reference computes, per (b,c) tree, S[n] = sum of s[k] over the
  root->n path (pointer-doubling with K=12 covers depth < 4096; actual
  random-recursive-tree depth is ~35), with
      s[k] = gate[k] * (level[k] - level[parent[k]]),  s[root] = level[root]
      gate[k] = (sigmoid(a_scaled - thr_norm) >= 0.5)  ==  (attr[k] >= thr)
  (min-max scaling is strictly monotone, so the 0.5-sigmoid threshold
  reduces exactly to the raw comparison), then out[pix] = S[node[pix]].

  Path sums over a tree are an Euler-tour prefix scan: entering node k adds
  s[k], leaving subtracts it; the running sum at k's entry event equals
  S[k].  Leaf exit events are elided (the stream shrinks 2N -> ~1.5N): a
  leaf's entry slot carries its attr with the SIGN BIT flipped, so the scan
  gate (attr >= thr) reads 0 there (no carry pollution) while a second gate
  (attr <= -thr) recovers the leaf's own contribution in a post-scan add:

      out[j] = inclusive_scan(w2)[j] + (cross_partition_carry + root_level)
               + (attr[j] <= -thr) * w1[j]
      w2[j]  = (attr[j] >= thr) * w1[j],   w1[j] = lv[j] - plv[j]

  The host derives the (data-independent) tour layout from the int32
  `parent` tensor alone; the device does all floating-point math: gates,
  residues, the ~393k-element prefix scan per tree (per-partition scan +
  PE-matmul cross-partition carry), fully dense — no data-dependent
  addressing on device.

Precision: level payloads travel as fp16 (entry/exit contribution pairs are
exact fp16 negations — swapped operands — so path-sum error grows only with
tree depth ~35, not stream length).  attr stays fp32: the gate compare must
not flip near the threshold.  The scan state is fp32 in hardware regardless
of operand dtype; only the stored output rounds to fp16.

Engine placement (HW-tuned): DVE runs w1/w2/the carry-seeded scan and two
of the three final adds; one tree's final add is striped onto GPSIMD (slow,
but idle and off the critical chain); the leaf gate (Sign+Relu of
sign-flipped attr) runs on the otherwise-idle Activation engine; the
cross-partition carry is a strict-upper-triangular-ones matmul on the
otherwise-idle PE (its result seeds the scan's initial value).
Loads issue from SP and the result store from Activation so the transfers
overlap engine-side overheads (the DMA bus itself serializes ~370GB/s).

Sharding: trees are independent per (b,c); the 24 trees go 3-per-NeuronCore
across 8 cores (data parallel, zero cross-device communication).

Host does ONLY integer index planning (from `parent` / `pixel_to_node`) and
data marshaling (reordering input copies into event order, sign-bit flips
on the uint32 view, inverse map on the returned scan); every floating-point
operation on attr/level/thr values runs on the NeuronCores.
"""

import numpy as np

P = 128            # SBUF partitions
TREES_PER_CORE = 3
N_CORES = 8
LEVEL_DTYPE = np.float16  # u8 halves DMA but the DVE u8 path is slower on HW

_CACHE = {}


# ----------------------------------------------------------------------------
# Host-side integer planning (uses only `parent` / `pixel_to_node`)
# ----------------------------------------------------------------------------

def _tree_plan(parent):
    """parent: (N,) int with parent[n] < n for n >= 1.

    Returns ev_enter (N,) int64: position of each node's entry event in the
    2N-long Euler event stream.  Root (node 0) is excluded from the stream;
    positions 0 and 2N-1 are zero-contribution pads, and ev_enter[0] = 0
    (the running sum there is 0; the root's base level is added globally).
    """
    N = parent.shape[0]
    par = parent.astype(np.int64)
    ar = np.arange(N)

    # depth (= #edges to root) via pointer doubling with absorbing root
    val = (ar != 0).astype(np.int64)
    a = par.copy()
    a[0] = 0
    for _ in range(20):
        if not a.any():
            break
        val = val + val[a]
        a = a[a]
    depth = val
    maxd = int(depth.max())
    if maxd >= 4096:
        return None, None, maxd

    # subtree sizes, bottom-up by depth level
    size = np.ones(N, np.int64)
    order = np.argsort(depth, kind="stable")
    bounds = np.searchsorted(depth[order], np.arange(maxd + 2))
    for d in range(maxd, 0, -1):
        nodes = order[bounds[d]:bounds[d + 1]]
        if len(nodes) == 0:
            continue
        size += np.bincount(par[nodes], weights=size[nodes],
                            minlength=N).astype(np.int64)

    # prefix of earlier-sibling subtree sizes (children visited in index order)
    sibord = np.argsort(par[1:], kind="stable") + 1
    sz = size[sibord]
    cs = np.cumsum(sz) - sz
    pgroup = par[sibord]
    first = np.ones(len(sibord), bool)
    first[1:] = pgroup[1:] != pgroup[:-1]
    base = np.where(first, cs, 0)
    np.maximum.accumulate(base, out=base)
    bss = np.zeros(N, np.int64)
    bss[sibord] = cs - base

    # preorder index = path-sum of (1 + bss) excluding root, via doubling
    c = 1 + bss
    c[0] = 0
    S = c
    a = par.copy()
    a[0] = 0
    for _ in range(20):
        if not a.any():
            break
        S = S + S[a]
        a = a[a]
    pre = S
    ev_enter = 2 * pre - depth
    ev_enter[0] = 0
    return ev_enter, size, maxd


def _host_preprocess(attr, level, thr, parent, pixel_to_node):
    """Returns (in_maps for 8 cores, q (T, HW) int32 slot positions, F)."""
    B, C, N = attr.shape
    T = B * C
    twoN = 2 * N
    attr2 = np.ascontiguousarray(attr.reshape(T, N))
    level2 = np.ascontiguousarray(level.reshape(T, N))
    par2 = np.ascontiguousarray(parent.reshape(T, N))
    pix2 = pixel_to_node.reshape(T, -1)

    # pass 1: plan all trees, find the common padded slot count
    plans = []
    maxM = 0
    nr = np.arange(1, N)
    for t in range(T):
        ev_enter, size, maxd = _tree_plan(par2[t])
        if maxd >= 4096:
            # reference's K=12 pointer doubling truncates paths longer than
            # 4096; the Euler scan computes the untruncated sum -> not
            # equivalent. Caller must use the exact fallback.
            return None, None, None
        ev_exit = ev_enter + 2 * size - 1
        pr = par2[t]
        nch = np.bincount(pr[1:], minlength=N)
        leaf = nch == 0
        keep = np.ones(twoN, bool)
        keep[ev_exit[leaf]] = False    # drop leaf exits
        keep[twoN - 1] = False         # drop trailing root pad
        newpos = (np.cumsum(keep) - 1).astype(np.int64)
        M = int(newpos[-1] + 1)
        maxM = max(maxM, M)
        plans.append((ev_enter, ev_exit, leaf, newpos))
    F = -(-maxM // (8 * P)) * 8        # slots per partition, padded to 8

    MP = P * F
    evA = np.zeros((T, MP), np.float32)
    evL = np.zeros((T, 2 * MP), LEVEL_DTYPE)   # per row: [lv | plv]
    q = np.empty((T, pix2.shape[1]), np.int32)
    for t in range(T):
        ev_enter, ev_exit, leaf, newpos = plans[t]
        at, lv, pr = attr2[t], level2[t], par2[t]
        en2 = newpos[ev_enter]
        ex2 = newpos[ev_exit]
        plv = lv[pr[nr]]
        ni = nr[~leaf[1:]]             # internal non-root nodes
        nl = nr[leaf[1:]]              # leaf nodes
        evA[t, en2[ni]] = at[ni]
        evA[t, ex2[ni]] = at[ni]
        afl = at[nl].copy()
        afl.view(np.uint32)[:] ^= 0x80000000   # sign-bit flip (integer op)
        evA[t, en2[nl]] = afl
        el = evL[t, :MP]
        ep = evL[t, MP:]
        el[en2[nr]] = lv[nr]
        ep[en2[nr]] = plv
        el[ex2[ni]] = plv[~leaf[1:]]   # swapped operands => exact negation
        ep[ex2[ni]] = lv[ni]
        q[t] = en2[np.clip(pix2[t], 0, N - 1)].astype(np.int32)

    thr_f = np.float32(thr.reshape(-1)[0])
    negthr = np.array(thr_f, np.float32)
    negthr.view(np.uint32)[...] ^= 0x80000000   # sign-bit flip (integer op)
    tri = np.triu(np.ones((P, P), np.float32), 1)   # tri[k,m]=1 iff k<m
    in_maps = []
    for c in range(N_CORES):
        tt = slice(c * TREES_PER_CORE, (c + 1) * TREES_PER_CORE)
        params = np.zeros((P, 4 * TREES_PER_CORE), np.float32)
        for k in range(TREES_PER_CORE):
            params[:, 4 * k] = thr_f
            params[:, 4 * k + 1] = negthr
            params[:, 4 * k + 2] = level2[c * TREES_PER_CORE + k, 0]
        in_maps.append({
            "evA": evA[tt].reshape(TREES_PER_CORE * P, F),
            "evL": evL[tt].reshape(TREES_PER_CORE, 2, P, F)
                .transpose(0, 2, 1, 3).reshape(TREES_PER_CORE * P, 2 * F),
            "params": params, "tri": tri})
    return in_maps, q, F


# ----------------------------------------------------------------------------
# Device program
# ----------------------------------------------------------------------------

def _build_nc(F, repeat=1, cfg=None):
    import concourse.bacc as bacc
    import concourse.mybir as mybir
    import concourse.tile as tile
    import concourse.bass as bass

    cfg = cfg or {}
    bufs = cfg.get("bufs", 2)
    version = cfg.get("version", "v3")   # "v3" | "v4"
    w1_eng = cfg.get("w1_eng", "vector")
    gneg_eng = cfg.get("gneg_eng", "act")  # v4: "act" | "dve"
    scan_bypass = cfg.get("scan_bypass", True)
    o1_via_act = cfg.get("o1_via_act", True)
    carry_in_scan = cfg.get("carry_in_scan", True)
    finadd_pool_stripe = cfg.get("finadd_pool_stripe", 1)  # trees on Pool
    o1_pool_stripe = cfg.get("o1_pool_stripe", 0)
    in_bufs = cfg.get("in_bufs", None)
    noscan = cfg.get("noscan", False)
    odma = cfg.get("odma", "scalar")     # "scalar" | "gpsimd" | "sync"
    mode = cfg.get("mode", "full")       # "full" | "dmaonly"

    f32 = mybir.dt.float32
    f16 = mybir.dt.float16
    op = mybir.AluOpType
    TP = TREES_PER_CORE * P

    nc = bacc.Bacc("TRN2", target_bir_lowering=False, debug=False,
                   num_devices=N_CORES)
    evA = nc.dram_tensor("evA", [TP, F], f32, kind="ExternalInput")
    lvdt = {np.dtype(np.uint8): mybir.dt.uint8,
            np.dtype(np.float16): f16}[np.dtype(LEVEL_DTYPE)]
    evL = nc.dram_tensor("evL", [TP, 2 * F], lvdt, kind="ExternalInput")
    params = nc.dram_tensor("params", [P, 4 * TREES_PER_CORE], f32,
                            kind="ExternalInput")
    triT = nc.dram_tensor("tri", [P, P], f32, kind="ExternalInput")
    Rout = nc.dram_tensor("R", [TP, F], f16, kind="ExternalOutput")

    with tile.TileContext(nc) as tc:
        with tc.tile_pool(name="sbuf", bufs=bufs) as pool, \
             tc.tile_pool(name="psum", space=bass.MemorySpace.PSUM,
                          bufs=2) as ppool:
            zero16 = pool.tile([P, 1], f16, tag="z16", bufs=1)
            nc.vector.memset(zero16[:], 0.0)
            prm = pool.tile([P, 4 * TREES_PER_CORE], f32, tag="prm", bufs=1)
            nc.sync.dma_start(prm, params.ap()[:, :])
            tri = pool.tile([P, P], f32, tag="tri", bufs=1)
            nc.sync.dma_start(tri, triT.ap()[:, :])
            odma_eng = {"scalar": nc.scalar, "gpsimd": nc.gpsimd,
                        "sync": nc.sync}[odma]
            for i, t in enumerate([tt % TREES_PER_CORE for tt in
                                   range(TREES_PER_CORE * repeat)]):
                rows = slice(t * P, (t + 1) * P)
                ea = pool.tile([P, F], f32, tag="ea",
                               **({"bufs": in_bufs} if in_bufs else {}))
                nc.sync.dma_start(ea, evA.ap()[rows, :])
                el = pool.tile([P, 2 * F], lvdt, tag="el",
                               **({"bufs": in_bufs} if in_bufs else {}))
                nc.sync.dma_start(el, evL.ap()[rows, :])
                if mode == "dmaonly":
                    odma_eng.dma_start(Rout.ap()[rows, :],
                                       el[:, 0:F])
                    continue

                # w1 = level - parent_level (exact fp16 negation pairs)
                w1_e = {"gpsimd": nc.gpsimd, "vector": nc.vector}[w1_eng]
                w1 = pool.tile([P, F], f16, tag="w1")
                w1_e.tensor_tensor(out=w1[:], in0=el[:, 0:F],
                                   in1=el[:, F:2 * F], op=op.subtract)
                # w2 = (attr >= thr) * w1 with fused per-partition row sums
                w2 = pool.tile([P, F], f16, tag="w2")
                rowsum = pool.tile([P, 1], f32, tag="rowsum")
                nc.vector.scalar_tensor_tensor(
                    out=w2[:], in0=ea[:], scalar=prm[:, 4 * t:4 * t + 1],
                    in1=w1[:], op0=op.is_ge, op1=op.mult,
                    accum_out=rowsum[:])

                # cross-partition exclusive prefix of rowsums on the (idle)
                # PE: excl[p] = sum_{k<p} rowsum[k] via strict-upper ones
                excl = ppool.tile([P, 1], f32, tag="excl")
                nc.tensor.matmul(excl[:], tri[:], rowsum[:])
                carry2 = pool.tile([P, 1], f32, tag="carry2")
                nc.vector.tensor_tensor(out=carry2[:], in0=excl[:],
                                        in1=prm[:, 4 * t + 2:4 * t + 3],
                                        op=op.add)

                ot = pool.tile([P, F], f16, tag="ot")
                if version == "v3":
                    # o1 = (attr <= -thr)*w1; carry-seeded scan; final
                    # 3-operand add on DVE
                    o1 = pool.tile([P, F], f16, tag="o1")
                    if o1_via_act:
                        sg = pool.tile([P, F], f16, tag="sg")
                        nc.scalar.activation(
                            sg[:], ea[:], mybir.ActivationFunctionType.Sign,
                            bias=prm[:, 4 * t + 1:4 * t + 2], scale=-1.0)
                        gneg = pool.tile([P, F], f16, tag="gneg")
                        nc.scalar.activation(
                            gneg[:], sg[:], mybir.ActivationFunctionType.Relu)
                        o1_e = (nc.gpsimd
                                if t >= TREES_PER_CORE - int(o1_pool_stripe)
                                else nc.vector)
                        o1_e.tensor_tensor(out=o1[:], in0=gneg[:],
                                           in1=w1[:], op=op.mult)
                    else:
                        nc.vector.scalar_tensor_tensor(
                            out=o1[:], in0=ea[:],
                            scalar=prm[:, 4 * t + 1:4 * t + 2],
                            in1=w1[:], op0=op.is_le, op1=op.mult)
                    rf = pool.tile([P, F], f16, tag="rf")
                    seed = carry2[:, 0:1] if carry_in_scan else 0.0
                    if noscan:
                        nc.vector.tensor_tensor(out=rf[:], in0=w2[:],
                                                in1=o1[:], op=op.add)
                    elif scan_bypass:
                        nc.vector.tensor_tensor_scan(
                            out=rf[:], data0=w2[:], data1=w2[:],
                            initial=seed, op0=op.add, op1=op.bypass)
                    else:
                        nc.vector.tensor_tensor_scan(
                            out=rf[:], data0=w2[:],
                            data1=zero16[:].to_broadcast([P, F]),
                            initial=seed, op0=op.add, op1=op.add)
                    if carry_in_scan:
                        fin_e = (nc.gpsimd
                                 if t >= TREES_PER_CORE - int(finadd_pool_stripe)
                                 else nc.vector)
                        fin_e.tensor_tensor(out=ot[:], in0=rf[:],
                                            in1=o1[:], op=op.add)
                    else:
                        nc.vector.scalar_tensor_tensor(
                            out=ot[:], in0=rf[:], scalar=carry2[:, 0:1],
                            in1=o1[:], op0=op.add, op1=op.add)
                else:
                    # gneg = (attr <= -thr) as 0/1 (leaf slots carry
                    # sign-flipped attr, so Sign(-attr-thr) is +1 there)
                    gneg = pool.tile([P, F], f16, tag="gneg")
                    if gneg_eng == "act":
                        sg = pool.tile([P, F], f16, tag="sg")
                        nc.scalar.activation(
                            sg[:], ea[:], mybir.ActivationFunctionType.Sign,
                            bias=prm[:, 4 * t + 1:4 * t + 2], scale=-1.0)
                        nc.scalar.activation(
                            gneg[:], sg[:], mybir.ActivationFunctionType.Relu)
                    else:
                        nc.vector.tensor_scalar(
                            out=gneg[:], in0=ea[:],
                            scalar1=prm[:, 4 * t + 1:4 * t + 2],
                            scalar2=None, op0=op.is_le)
                    # o1 = gneg * w1: the leaf contributions
                    o1 = pool.tile([P, F], f16, tag="o1")
                    nc.gpsimd.tensor_tensor(out=o1[:], in0=gneg[:],
                                            in1=w1[:], op=op.mult)
                    # d0 = w2 - shift1(o1): with data1=o1 below, the scan
                    # emits inclscan(w2)[j] + o1[j] directly (telescoping).
                    # f32: the fp16-fp16 difference must stay exact or the
                    # telescoping drifts over the 393k-slot stream
                    d0 = pool.tile([P, F], f32, tag="d0")
                    nc.gpsimd.tensor_copy(d0[:, 0:1], w2[:, 0:1])
                    nc.gpsimd.tensor_tensor(out=d0[:, 1:F], in0=w2[:, 1:F],
                                            in1=o1[:, 0:F - 1],
                                            op=op.subtract)
                    # the scan IS the final output: fp32 state, fp16 out
                    if noscan:
                        nc.vector.tensor_tensor(out=ot[:], in0=w2[:],
                                                in1=o1[:], op=op.add)
                    else:
                        nc.vector.tensor_tensor_scan(
                            out=ot[:], data0=d0[:], data1=o1[:],
                            initial=carry2[:, 0:1], op0=op.add, op1=op.add)
                odma_eng.dma_start(Rout.ap()[rows, :], ot[:])
    nc.compile()
    return nc


def _get_nc(F):
    key = ("nc", F)
    if key not in _CACHE:
        _CACHE[key] = _build_nc(F)
    return _CACHE[key]


# ----------------------------------------------------------------------------
# Fallback: exact f32 emulation of the reference (invalid/cyclic trees only)
# ----------------------------------------------------------------------------

def _fallback_reference(attr, level, thr, parent, pixel_to_node):
    B, C, N = attr.shape
    # replicate reference's scaled-sigmoid gate semantics
    amin = attr.min(-1, keepdims=True)
    amax = attr.max(-1, keepdims=True)
    denom = np.maximum(amax - amin, np.float32(1e-6))
    a_s = ((attr - amin) / denom).astype(np.float32)
    t_n = ((np.float32(thr.reshape(-1)[0]) - amin) / denom).astype(np.float32)
    d = (a_s - t_n).astype(np.float32)
    soft = (1.0 / (1.0 + np.exp(-d.astype(np.float64)))).astype(np.float32)
    gate = (soft >= 0.5).astype(np.float32)
    pixel_to_node = np.clip(pixel_to_node, 0, N - 1)
    pl = np.take_along_axis(level, np.clip(parent, 0, N - 1).astype(np.int64),
                            axis=-1)
    s = gate * (level - pl)
    s[..., 0] = level[..., 0]
    s = np.concatenate([s, np.zeros((B, C, 1), np.float32)], axis=-1)
    p = np.concatenate([np.clip(parent, 0, N).astype(np.int32),
                        np.full((B, C, 1), N, np.int32)], axis=-1)
    p[..., 0] = N
    S = s.astype(np.float32)
    pp = p.astype(np.int64)
    for _ in range(12):
        S = (S + np.take_along_axis(S, pp, axis=-1)).astype(np.float32)
        pp = np.take_along_axis(pp, pp, axis=-1)
    S = S[..., :N]
    out = np.take_along_axis(S, pixel_to_node.astype(np.int64), axis=-1)
    HW = pixel_to_node.shape[-1]
    H = int(np.sqrt(HW))
    return out.reshape(B, C, H, HW // H).astype(np.float32)


# ----------------------------------------------------------------------------
# Entry point
# ----------------------------------------------------------------------------

def kernel(attr, level, thr_raw, parent, pixel_to_node):
    attr = np.asarray(attr, np.float32)
    level = np.asarray(level, np.float32)
    thr_raw = np.asarray(thr_raw, np.float32)
    parent = np.asarray(parent)
    pixel_to_node = np.asarray(pixel_to_node)
    B, C, N = attr.shape
    HW = pixel_to_node.shape[-1]
    H = int(np.sqrt(HW))

    par2 = parent.reshape(-1, N)
    valid = bool(np.all(par2[:, 1:] < np.arange(1, N)) and np.all(par2 >= 0)
                 and float(thr_raw.reshape(-1)[0]) > 0.0)
    if not valid or B * C != N_CORES * TREES_PER_CORE:
        return _fallback_reference(attr, level, thr_raw, parent, pixel_to_node)

    in_maps, q, F = _host_preprocess(attr, level, thr_raw, parent,
                                     pixel_to_node)
    if in_maps is None:  # depth >= 4096: doubling truncation applies
        return _fallback_reference(attr, level, thr_raw, parent,
                                   pixel_to_node)
    try:
        nc = _get_nc(F)
        from concourse.bass_utils import run_bass_kernel_spmd
        res = run_bass_kernel_spmd(nc, in_maps, core_ids=list(range(N_CORES)))
    except Exception as e:  # infra failure: still return a correct result
        import traceback
        traceback.print_exc()
        print(f"kernel: device path failed ({type(e).__name__}); "
              "falling back to host emulation")
        return _fallback_reference(attr, level, thr_raw, parent,
                                   pixel_to_node)

    out = np.empty((B * C, HW), np.float32)
    for c in range(N_CORES):
        R = res.results[c]["R"].astype(np.float32).reshape(TREES_PER_CORE,
                                                           P * F)
        for k in range(TREES_PER_CORE):
            t = c * TREES_PER_CORE + k
            out[t] = R[k][q[t]]
    return out.reshape(B, C, H, HW // H)


# revision 33
# speedup vs baseline: 2.5272x; 1.0325x over previous
"""Trainium2 kernel for nn_ConnectedThresholdLayer (gated connected-filter on
morphological max-trees + pixel reconstruction).

Mathematical reformulation (exactly equivalent to the reference on valid
trees, which setup_inputs always produces):

  The reference computes, per (b,c) tree, S[n] = sum of s[k] over the
  root->n path (pointer-doubling with K=12 covers depth < 4096; actual
  random-recursive-tree depth is ~35), with
      s[k] = gate[k] * (level[k] - level[parent[k]]),  s[root] = level[root]
      gate[k] = (sigmoid(a_scaled - thr_norm) >= 0.5)  ==  (attr[k] >= thr)
  (min-max scaling is strictly monotone, so the 0.5-sigmoid threshold
  reduces exactly to the raw comparison), then out[pix] = S[node[pix]].

  Path sums over a tree are an Euler-tour prefix scan: entering node k adds
  s[k], leaving subtracts it; the running sum at k's entry event equals
  S[k].  Leaf exit events are elided (the stream shrinks 2N -> ~1.5N): a
  leaf's entry slot carries its attr with the SIGN BIT flipped, so the scan
  gate (attr >= thr) reads 0 there (no carry pollution) while a second gate
  (attr <= -thr) recovers the leaf's own contribution in a post-scan add:

      out[j] = inclusive_scan(w2)[j] + (cross_partition_carry + root_level)
               + (attr[j] <= -thr) * w1[j]
      w2[j]  = (attr[j] >= thr) * w1[j],   w1[j] = lv[j] - plv[j]

  The host derives the (data-independent) tour layout from the int32
  `parent` tensor alone; the device does all floating-point math: gates,
  residues, the ~393k-element prefix scan per tree (per-partition scan +
  PE-matmul cross-partition carry), fully dense — no data-dependent
  addressing on device.

Precision: level payloads travel as fp16 (entry/exit contribution pairs are
exact fp16 negations — swapped operands — so path-sum error grows only with
tree depth ~35, not stream length).  attr stays fp32: the gate compare must
not flip near the threshold.  The scan state is fp32 in hardware regardless
of operand dtype; only the stored output rounds to fp16.

Engine placement (HW-tuned): DVE runs w1/w2/the carry-seeded scan and two
of the three final adds; one tree's final add is striped onto GPSIMD (slow,
but idle and off the critical chain); the leaf gate (Sign+Relu of
sign-flipped attr) runs on the otherwise-idle Activation engine; the
cross-partition carry is a strict-upper-triangular-ones matmul on the
otherwise-idle PE (its result seeds the scan's initial value).
Loads issue from SP and the result store from Activation so the transfers
overlap engine-side overheads (the DMA bus itself serializes ~370GB/s).

Sharding: trees are independent per (b,c); the 24 trees go 3-per-NeuronCore
across 8 cores (data parallel, zero cross-device communication).

Host does ONLY integer index planning (from `parent` / `pixel_to_node`) and
data marshaling (reordering input copies into event order, sign-bit flips
on the uint32 view, inverse map on the returned scan); every floating-point
operation on attr/level/thr values runs on the NeuronCores.
"""

import numpy as np

P = 128            # SBUF partitions
TREES_PER_CORE = 3
N_CORES = 8
LEVEL_DTYPE = np.float16  # u8 halves DMA but the DVE u8 path is slower on HW

_CACHE = {}


# ----------------------------------------------------------------------------
# Host-side integer planning (uses only `parent` / `pixel_to_node`)
# ----------------------------------------------------------------------------

def _tree_plan(parent):
    """parent: (N,) int with parent[n] < n for n >= 1.

    Returns ev_enter (N,) int64: position of each node's entry event in the
    2N-long Euler event stream.  Root (node 0) is excluded from the stream;
    positions 0 and 2N-1 are zero-contribution pads, and ev_enter[0] = 0
    (the running sum there is 0; the root's base level is added globally).
    """
    N = parent.shape[0]
    par = parent.astype(np.int64)
    ar = np.arange(N)

    # depth (= #edges to root) via pointer doubling with absorbing root
    val = (ar != 0).astype(np.int64)
    a = par.copy()
    a[0] = 0
    for _ in range(20):
        if not a.any():
            break
        val = val + val[a]
        a = a[a]
    depth = val
    maxd = int(depth.max())
    if maxd >= 4096:
        return None, None, maxd

    # subtree sizes, bottom-up by depth level
    size = np.ones(N, np.int64)
    order = np.argsort(depth, kind="stable")
    bounds = np.searchsorted(depth[order], np.arange(maxd + 2))
    for d in range(maxd, 0, -1):
        nodes = order[bounds[d]:bounds[d + 1]]
        if len(nodes) == 0:
            continue
        size += np.bincount(par[nodes], weights=size[nodes],
                            minlength=N).astype(np.int64)

    # prefix of earlier-sibling subtree sizes (children visited in index order)
    sibord = np.argsort(par[1:], kind="stable") + 1
    sz = size[sibord]
    cs = np.cumsum(sz) - sz
    pgroup = par[sibord]
    first = np.ones(len(sibord), bool)
    first[1:] = pgroup[1:] != pgroup[:-1]
    base = np.where(first, cs, 0)
    np.maximum.accumulate(base, out=base)
    bss = np.zeros(N, np.int64)
    bss[sibord] = cs - base

    # preorder index = path-sum of (1 + bss) excluding root, via doubling
    c = 1 + bss
    c[0] = 0
    S = c
    a = par.copy()
    a[0] = 0
    for _ in range(20):
        if not a.any():
            break
        S = S + S[a]
        a = a[a]
    pre = S
    ev_enter = 2 * pre - depth
    ev_enter[0] = 0
    return ev_enter, size, maxd


def _host_preprocess(attr, level, thr, parent, pixel_to_node):
    """Returns (in_maps for 8 cores, q (T, HW) int32 slot positions, F)."""
    B, C, N = attr.shape
    T = B * C
    twoN = 2 * N
    attr2 = np.ascontiguousarray(attr.reshape(T, N))
    level2 = np.ascontiguousarray(level.reshape(T, N))
    par2 = np.ascontiguousarray(parent.reshape(T, N))
    pix2 = pixel_to_node.reshape(T, -1)

    # pass 1: plan all trees, find the common padded slot count
    plans = []
    maxM = 0
    nr = np.arange(1, N)
    for t in range(T):
        ev_enter, size, maxd = _tree_plan(par2[t])
        if maxd >= 4096:
            # reference's K=12 pointer doubling truncates paths longer than
            # 4096; the Euler scan computes the untruncated sum -> not
            # equivalent. Caller must use the exact fallback.
            return None, None, None
        ev_exit = ev_enter + 2 * size - 1
        pr = par2[t]
        nch = np.bincount(pr[1:], minlength=N)
        leaf = nch == 0
        keep = np.ones(twoN, bool)
        keep[ev_exit[leaf]] = False    # drop leaf exits
        keep[twoN - 1] = False         # drop trailing root pad
        newpos = (np.cumsum(keep) - 1).astype(np.int64)
        M = int(newpos[-1] + 1)
        maxM = max(maxM, M)
        plans.append((ev_enter, ev_exit, leaf, newpos))
    F = -(-maxM // (8 * P)) * 8        # slots per partition, padded to 8

    MP = P * F
    evA = np.zeros((T, MP), np.float32)
    evL = np.zeros((T, 2 * MP), LEVEL_DTYPE)   # per row: [lv | plv]
    q = np.empty((T, pix2.shape[1]), np.int32)
    for t in range(T):
        ev_enter, ev_exit, leaf, newpos = plans[t]
        at, lv, pr = attr2[t], level2[t], par2[t]
        en2 = newpos[ev_enter]
        ex2 = newpos[ev_exit]
        plv = lv[pr[nr]]
        ni = nr[~leaf[1:]]             # internal non-root nodes
        nl = nr[leaf[1:]]              # leaf nodes
        evA[t, en2[ni]] = at[ni]
        evA[t, ex2[ni]] = at[ni]
        afl = at[nl].copy()
        afl.view(np.uint32)[:] ^= 0x80000000   # sign-bit flip (integer op)
        evA[t, en2[nl]] = afl
        el = evL[t, :MP]
        ep = evL[t, MP:]
        el[en2[nr]] = lv[nr]
        ep[en2[nr]] = plv
        el[ex2[ni]] = plv[~leaf[1:]]   # swapped operands => exact negation
        ep[ex2[ni]] = lv[ni]
        q[t] = en2[np.clip(pix2[t], 0, N - 1)].astype(np.int32)

    thr_f = np.float32(thr.reshape(-1)[0])
    negthr = np.array(thr_f, np.float32)
    negthr.view(np.uint32)[...] ^= 0x80000000   # sign-bit flip (integer op)
    tri = np.triu(np.ones((P, P), np.float32), 1)   # tri[k,m]=1 iff k<m
    in_maps = []
    for c in range(N_CORES):
        tt = slice(c * TREES_PER_CORE, (c + 1) * TREES_PER_CORE)
        params = np.zeros((P, 4 * TREES_PER_CORE), np.float32)
        for k in range(TREES_PER_CORE):
            params[:, 4 * k] = thr_f
            params[:, 4 * k + 1] = negthr
            params[:, 4 * k + 2] = level2[c * TREES_PER_CORE + k, 0]
        in_maps.append({
            "evA": evA[tt].reshape(TREES_PER_CORE * P, F),
            "evL": evL[tt].reshape(TREES_PER_CORE, 2, P, F)
                .transpose(0, 2, 1, 3).reshape(TREES_PER_CORE * P, 2 * F),
            "params": params, "tri": tri})
    return in_maps, q, F


# ----------------------------------------------------------------------------
# Device program
# ----------------------------------------------------------------------------

def _build_nc(F, repeat=1, cfg=None):
    import concourse.bacc as bacc
    import concourse.mybir as mybir
    import concourse.tile as tile
    import concourse.bass as bass

    cfg = cfg or {}
    bufs = cfg.get("bufs", 2)
    version = cfg.get("version", "v3")   # "v3" | "v4"
    w1_eng = cfg.get("w1_eng", "vector")
    gneg_eng = cfg.get("gneg_eng", "act")  # v4: "act" | "dve"
    scan_bypass = cfg.get("scan_bypass", True)
    o1_via_act = cfg.get("o1_via_act", True)
    carry_in_scan = cfg.get("carry_in_scan", True)
    finadd_pool_stripe = cfg.get("finadd_pool_stripe", 1)  # trees on Pool
    o1_pool_stripe = cfg.get("o1_pool_stripe", 0)
    pipelined_store = cfg.get("pipelined_store", True)
    in_bufs = cfg.get("in_bufs", None)
    noscan = cfg.get("noscan", False)
    odma = cfg.get("odma", "scalar")     # "scalar" | "gpsimd" | "sync"
    mode = cfg.get("mode", "full")       # "full" | "dmaonly"

    f32 = mybir.dt.float32
    f16 = mybir.dt.float16
    op = mybir.AluOpType
    TP = TREES_PER_CORE * P

    nc = bacc.Bacc("TRN2", target_bir_lowering=False, debug=False,
                   num_devices=N_CORES)
    evA = nc.dram_tensor("evA", [TP, F], f32, kind="ExternalInput")
    lvdt = {np.dtype(np.uint8): mybir.dt.uint8,
            np.dtype(np.float16): f16}[np.dtype(LEVEL_DTYPE)]
    evL = nc.dram_tensor("evL", [TP, 2 * F], lvdt, kind="ExternalInput")
    params = nc.dram_tensor("params", [P, 4 * TREES_PER_CORE], f32,
                            kind="ExternalInput")
    triT = nc.dram_tensor("tri", [P, P], f32, kind="ExternalInput")
    Rout = nc.dram_tensor("R", [TP, F], f16, kind="ExternalOutput")

    with tile.TileContext(nc) as tc:
        with tc.tile_pool(name="sbuf", bufs=bufs) as pool, \
             tc.tile_pool(name="psum", space=bass.MemorySpace.PSUM,
                          bufs=2) as ppool:
            zero16 = pool.tile([P, 1], f16, tag="z16", bufs=1)
            nc.vector.memset(zero16[:], 0.0)
            prm = pool.tile([P, 4 * TREES_PER_CORE], f32, tag="prm", bufs=1)
            nc.sync.dma_start(prm, params.ap()[:, :])
            tri = pool.tile([P, P], f32, tag="tri", bufs=1)
            nc.sync.dma_start(tri, triT.ap()[:, :])
            odma_eng = {"scalar": nc.scalar, "gpsimd": nc.gpsimd,
                        "sync": nc.sync}[odma]
            pending_store = None
            for i, t in enumerate([tt % TREES_PER_CORE for tt in
                                   range(TREES_PER_CORE * repeat)]):
                rows = slice(t * P, (t + 1) * P)
                ea = pool.tile([P, F], f32, tag="ea",
                               **({"bufs": in_bufs} if in_bufs else {}))
                nc.sync.dma_start(ea, evA.ap()[rows, :])
                el = pool.tile([P, 2 * F], lvdt, tag="el",
                               **({"bufs": in_bufs} if in_bufs else {}))
                nc.sync.dma_start(el, evL.ap()[rows, :])
                if mode == "dmaonly":
                    odma_eng.dma_start(Rout.ap()[rows, :],
                                       el[:, 0:F])
                    continue

                # Act first (leaf gate needs only ea), so on Act's in-order
                # queue this tree's Sign/Relu precede the previous tree's
                # (late-ready) result store
                if version == "v3" and o1_via_act:
                    sg = pool.tile([P, F], f16, tag="sg")
                    nc.scalar.activation(
                        sg[:], ea[:], mybir.ActivationFunctionType.Sign,
                        bias=prm[:, 4 * t + 1:4 * t + 2], scale=-1.0)
                    gneg = pool.tile([P, F], f16, tag="gneg")
                    nc.scalar.activation(
                        gneg[:], sg[:], mybir.ActivationFunctionType.Relu)
                if pipelined_store and pending_store is not None:
                    odma_eng.dma_start(*pending_store)
                    pending_store = None

                # w1 = level - parent_level (exact fp16 negation pairs)
                w1_e = {"gpsimd": nc.gpsimd, "vector": nc.vector}[w1_eng]
                w1 = pool.tile([P, F], f16, tag="w1")
                w1_e.tensor_tensor(out=w1[:], in0=el[:, 0:F],
                                   in1=el[:, F:2 * F], op=op.subtract)
                # w2 = (attr >= thr) * w1 with fused per-partition row sums
                w2 = pool.tile([P, F], f16, tag="w2")
                rowsum = pool.tile([P, 1], f32, tag="rowsum")
                nc.vector.scalar_tensor_tensor(
                    out=w2[:], in0=ea[:], scalar=prm[:, 4 * t:4 * t + 1],
                    in1=w1[:], op0=op.is_ge, op1=op.mult,
                    accum_out=rowsum[:])

                # cross-partition exclusive prefix of rowsums on the (idle)
                # PE: excl[p] = sum_{k<p} rowsum[k] via strict-upper ones
                excl = ppool.tile([P, 1], f32, tag="excl")
                nc.tensor.matmul(excl[:], tri[:], rowsum[:])
                carry2 = pool.tile([P, 1], f32, tag="carry2")
                nc.vector.tensor_tensor(out=carry2[:], in0=excl[:],
                                        in1=prm[:, 4 * t + 2:4 * t + 3],
                                        op=op.add)

                ot = pool.tile([P, F], f16, tag="ot")
                if version == "v3":
                    o1 = pool.tile([P, F], f16, tag="o1")
                    rf = pool.tile([P, F], f16, tag="rf")
                    seed = carry2[:, 0:1] if carry_in_scan else 0.0
                    # scan before o1 on DVE's in-order queue: the scan does
                    # not depend on Act's gneg, so it must not sit behind o1
                    if noscan:
                        pass
                    elif scan_bypass:
                        nc.vector.tensor_tensor_scan(
                            out=rf[:], data0=w2[:], data1=w2[:],
                            initial=seed, op0=op.add, op1=op.bypass)
                    else:
                        nc.vector.tensor_tensor_scan(
                            out=rf[:], data0=w2[:],
                            data1=zero16[:].to_broadcast([P, F]),
                            initial=seed, op0=op.add, op1=op.add)
                    # o1 = (attr <= -thr)*w1: the leaf contributions
                    if o1_via_act:
                        o1_e = (nc.gpsimd
                                if t >= TREES_PER_CORE - int(o1_pool_stripe)
                                else nc.vector)
                        o1_e.tensor_tensor(out=o1[:], in0=gneg[:],
                                           in1=w1[:], op=op.mult)
                    else:
                        nc.vector.scalar_tensor_tensor(
                            out=o1[:], in0=ea[:],
                            scalar=prm[:, 4 * t + 1:4 * t + 2],
                            in1=w1[:], op0=op.is_le, op1=op.mult)
                    if noscan:
                        nc.vector.tensor_tensor(out=rf[:], in0=w2[:],
                                                in1=o1[:], op=op.add)
                    if carry_in_scan:
                        fin_e = (nc.gpsimd
                                 if t >= TREES_PER_CORE - int(finadd_pool_stripe)
                                 else nc.vector)
                        fin_e.tensor_tensor(out=ot[:], in0=rf[:],
                                            in1=o1[:], op=op.add)
                    else:
                        nc.vector.scalar_tensor_tensor(
                            out=ot[:], in0=rf[:], scalar=carry2[:, 0:1],
                            in1=o1[:], op0=op.add, op1=op.add)
                else:
                    # gneg = (attr <= -thr) as 0/1 (leaf slots carry
                    # sign-flipped attr, so Sign(-attr-thr) is +1 there)
                    gneg = pool.tile([P, F], f16, tag="gneg")
                    if gneg_eng == "act":
                        sg = pool.tile([P, F], f16, tag="sg")
                        nc.scalar.activation(
                            sg[:], ea[:], mybir.ActivationFunctionType.Sign,
                            bias=prm[:, 4 * t + 1:4 * t + 2], scale=-1.0)
                        nc.scalar.activation(
                            gneg[:], sg[:], mybir.ActivationFunctionType.Relu)
                    else:
                        nc.vector.tensor_scalar(
                            out=gneg[:], in0=ea[:],
                            scalar1=prm[:, 4 * t + 1:4 * t + 2],
                            scalar2=None, op0=op.is_le)
                    # o1 = gneg * w1: the leaf contributions
                    o1 = pool.tile([P, F], f16, tag="o1")
                    nc.gpsimd.tensor_tensor(out=o1[:], in0=gneg[:],
                                            in1=w1[:], op=op.mult)
                    # d0 = w2 - shift1(o1): with data1=o1 below, the scan
                    # emits inclscan(w2)[j] + o1[j] directly (telescoping).
                    # f32: the fp16-fp16 difference must stay exact or the
                    # telescoping drifts over the 393k-slot stream
                    d0 = pool.tile([P, F], f32, tag="d0")
                    nc.gpsimd.tensor_copy(d0[:, 0:1], w2[:, 0:1])
                    nc.gpsimd.tensor_tensor(out=d0[:, 1:F], in0=w2[:, 1:F],
                                            in1=o1[:, 0:F - 1],
                                            op=op.subtract)
                    # the scan IS the final output: fp32 state, fp16 out
                    if noscan:
                        nc.vector.tensor_tensor(out=ot[:], in0=w2[:],
                                                in1=o1[:], op=op.add)
                    else:
                        nc.vector.tensor_tensor_scan(
                            out=ot[:], data0=d0[:], data1=o1[:],
                            initial=carry2[:, 0:1], op0=op.add, op1=op.add)
                if pipelined_store:
                    pending_store = (Rout.ap()[rows, :], ot[:])
                else:
                    odma_eng.dma_start(Rout.ap()[rows, :], ot[:])
            if pending_store is not None:
                odma_eng.dma_start(*pending_store)
    nc.compile()
    return nc


def _get_nc(F):
    key = ("nc", F)
    if key not in _CACHE:
        _CACHE[key] = _build_nc(F)
    return _CACHE[key]


# ----------------------------------------------------------------------------
# Fallback: exact f32 emulation of the reference (invalid/cyclic trees only)
# ----------------------------------------------------------------------------

def _fallback_reference(attr, level, thr, parent, pixel_to_node):
    B, C, N = attr.shape
    # replicate reference's scaled-sigmoid gate semantics
    amin = attr.min(-1, keepdims=True)
    amax = attr.max(-1, keepdims=True)
    denom = np.maximum(amax - amin, np.float32(1e-6))
    a_s = ((attr - amin) / denom).astype(np.float32)
    t_n = ((np.float32(thr.reshape(-1)[0]) - amin) / denom).astype(np.float32)
    d = (a_s - t_n).astype(np.float32)
    soft = (1.0 / (1.0 + np.exp(-d.astype(np.float64)))).astype(np.float32)
    gate = (soft >= 0.5).astype(np.float32)
    pixel_to_node = np.clip(pixel_to_node, 0, N - 1)
    pl = np.take_along_axis(level, np.clip(parent, 0, N - 1).astype(np.int64),
                            axis=-1)
    s = gate * (level - pl)
    s[..., 0] = level[..., 0]
    s = np.concatenate([s, np.zeros((B, C, 1), np.float32)], axis=-1)
    p = np.concatenate([np.clip(parent, 0, N).astype(np.int32),
                        np.full((B, C, 1), N, np.int32)], axis=-1)
    p[..., 0] = N
    S = s.astype(np.float32)
    pp = p.astype(np.int64)
    for _ in range(12):
        S = (S + np.take_along_axis(S, pp, axis=-1)).astype(np.float32)
        pp = np.take_along_axis(pp, pp, axis=-1)
    S = S[..., :N]
    out = np.take_along_axis(S, pixel_to_node.astype(np.int64), axis=-1)
    HW = pixel_to_node.shape[-1]
    H = int(np.sqrt(HW))
    return out.reshape(B, C, H, HW // H).astype(np.float32)


# ----------------------------------------------------------------------------
# Entry point
# ----------------------------------------------------------------------------

def kernel(attr, level, thr_raw, parent, pixel_to_node):
    attr = np.asarray(attr, np.float32)
    level = np.asarray(level, np.float32)
    thr_raw = np.asarray(thr_raw, np.float32)
    parent = np.asarray(parent)
    pixel_to_node = np.asarray(pixel_to_node)
    B, C, N = attr.shape
    HW = pixel_to_node.shape[-1]
    H = int(np.sqrt(HW))

    par2 = parent.reshape(-1, N)
    valid = bool(np.all(par2[:, 1:] < np.arange(1, N)) and np.all(par2 >= 0)
                 and float(thr_raw.reshape(-1)[0]) > 0.0)
    if not valid or B * C != N_CORES * TREES_PER_CORE:
        return _fallback_reference(attr, level, thr_raw, parent, pixel_to_node)

    in_maps, q, F = _host_preprocess(attr, level, thr_raw, parent,
                                     pixel_to_node)
    if in_maps is None:  # depth >= 4096: doubling truncation applies
        return _fallback_reference(attr, level, thr_raw, parent,
                                   pixel_to_node)
    try:
        nc = _get_nc(F)
        from concourse.bass_utils import run_bass_kernel_spmd
        res = run_bass_kernel_spmd(nc, in_maps, core_ids=list(range(N_CORES)))
    except Exception as e:  # infra failure: still return a correct result
        import traceback
        traceback.print_exc()
        print(f"kernel: device path failed ({type(e).__name__}); "
              "falling back to host emulation")
        return _fallback_reference(attr, level, thr_raw, parent,
                                   pixel_to_node)

    out = np.empty((B * C, HW), np.float32)
    for c in range(N_CORES):
        R = res.results[c]["R"].astype(np.float32).reshape(TREES_PER_CORE,
                                                           P * F)
        for k in range(TREES_PER_CORE):
            t = c * TREES_PER_CORE + k
            out[t] = R[k][q[t]]
    return out.reshape(B, C, H, HW // H)


# revision 35
# speedup vs baseline: 2.6647x; 1.0544x over previous
"""Trainium2 kernel for nn_ConnectedThresholdLayer (gated connected-filter on
morphological max-trees + pixel reconstruction).

Mathematical reformulation (exactly equivalent to the reference on valid
trees, which setup_inputs always produces):

  The reference computes, per (b,c) tree, S[n] = sum of s[k] over the
  root->n path (pointer-doubling with K=12 covers depth < 4096; actual
  random-recursive-tree depth is ~35), with
      s[k] = gate[k] * (level[k] - level[parent[k]]),  s[root] = level[root]
      gate[k] = (sigmoid(a_scaled - thr_norm) >= 0.5)  ==  (attr[k] >= thr)
  (min-max scaling is strictly monotone, so the 0.5-sigmoid threshold
  reduces exactly to the raw comparison), then out[pix] = S[node[pix]].

  Path sums over a tree are an Euler-tour prefix scan: entering node k adds
  s[k], leaving subtracts it; the running sum at k's entry event equals
  S[k].  Leaf exit events are elided (the stream shrinks 2N -> ~1.5N): a
  leaf's entry slot carries its attr with the SIGN BIT flipped, so the scan
  gate (attr >= thr) reads 0 there (no carry pollution) while a second gate
  (attr <= -thr) recovers the leaf's own contribution in a post-scan add:

      out[j] = inclusive_scan(w2)[j] + (cross_partition_carry + root_level)
               + (attr[j] <= -thr) * w1[j]
      w2[j]  = (attr[j] >= thr) * w1[j],   w1[j] = lv[j] - plv[j]

  The host derives the (data-independent) tour layout from the int32
  `parent` tensor alone; the device does all floating-point math: gates,
  residues, the ~393k-element prefix scan per tree (per-partition scan +
  PE-matmul cross-partition carry), fully dense — no data-dependent
  addressing on device.

Precision: level payloads travel as fp16 (entry/exit contribution pairs are
exact fp16 negations — swapped operands — so path-sum error grows only with
tree depth ~35, not stream length).  attr stays fp32: the gate compare must
not flip near the threshold.  The scan state is fp32 in hardware regardless
of operand dtype; only the stored output rounds to fp16.

Engine placement (HW-tuned): DVE runs w1/w2/the carry-seeded scan and two
of the three final adds; one tree's final add is striped onto GPSIMD (slow,
but idle and off the critical chain); the leaf gate (Sign+Relu of
sign-flipped attr) runs on the otherwise-idle Activation engine; the
cross-partition carry is a strict-upper-triangular-ones matmul on the
otherwise-idle PE (its result seeds the scan's initial value).
Loads issue from SP and the result store from Activation so the transfers
overlap engine-side overheads (the DMA bus itself serializes ~370GB/s).

Sharding: trees are independent per (b,c); the 24 trees go 3-per-NeuronCore
across 8 cores (data parallel, zero cross-device communication).

Host does ONLY integer index planning (from `parent` / `pixel_to_node`) and
data marshaling (reordering input copies into event order, sign-bit flips
on the uint32 view, inverse map on the returned scan); every floating-point
operation on attr/level/thr values runs on the NeuronCores.
"""

import numpy as np

P = 128            # SBUF partitions
TREES_PER_CORE = 3
N_CORES = 8
LEVEL_DTYPE = np.float16  # u8 halves DMA but the DVE u8 path is slower on HW

_CACHE = {}


# ----------------------------------------------------------------------------
# Host-side integer planning (uses only `parent` / `pixel_to_node`)
# ----------------------------------------------------------------------------

def _tree_plan(parent):
    """parent: (N,) int with parent[n] < n for n >= 1.

    Returns ev_enter (N,) int64: position of each node's entry event in the
    2N-long Euler event stream.  Root (node 0) is excluded from the stream;
    positions 0 and 2N-1 are zero-contribution pads, and ev_enter[0] = 0
    (the running sum there is 0; the root's base level is added globally).
    """
    N = parent.shape[0]
    par = parent.astype(np.int64)
    ar = np.arange(N)

    # depth (= #edges to root) via pointer doubling with absorbing root
    val = (ar != 0).astype(np.int64)
    a = par.copy()
    a[0] = 0
    for _ in range(20):
        if not a.any():
            break
        val = val + val[a]
        a = a[a]
    depth = val
    maxd = int(depth.max())
    if maxd >= 4096:
        return None, None, maxd

    # subtree sizes, bottom-up by depth level
    size = np.ones(N, np.int64)
    order = np.argsort(depth, kind="stable")
    bounds = np.searchsorted(depth[order], np.arange(maxd + 2))
    for d in range(maxd, 0, -1):
        nodes = order[bounds[d]:bounds[d + 1]]
        if len(nodes) == 0:
            continue
        size += np.bincount(par[nodes], weights=size[nodes],
                            minlength=N).astype(np.int64)

    # prefix of earlier-sibling subtree sizes (children visited in index order)
    sibord = np.argsort(par[1:], kind="stable") + 1
    sz = size[sibord]
    cs = np.cumsum(sz) - sz
    pgroup = par[sibord]
    first = np.ones(len(sibord), bool)
    first[1:] = pgroup[1:] != pgroup[:-1]
    base = np.where(first, cs, 0)
    np.maximum.accumulate(base, out=base)
    bss = np.zeros(N, np.int64)
    bss[sibord] = cs - base

    # preorder index = path-sum of (1 + bss) excluding root, via doubling
    c = 1 + bss
    c[0] = 0
    S = c
    a = par.copy()
    a[0] = 0
    for _ in range(20):
        if not a.any():
            break
        S = S + S[a]
        a = a[a]
    pre = S
    ev_enter = 2 * pre - depth
    ev_enter[0] = 0
    return ev_enter, size, maxd


def _host_preprocess(attr, level, thr, parent, pixel_to_node):
    """Returns (in_maps for 8 cores, q (T, HW) int32 slot positions, F)."""
    B, C, N = attr.shape
    T = B * C
    twoN = 2 * N
    attr2 = np.ascontiguousarray(attr.reshape(T, N))
    level2 = np.ascontiguousarray(level.reshape(T, N))
    par2 = np.ascontiguousarray(parent.reshape(T, N))
    pix2 = pixel_to_node.reshape(T, -1)

    # pass 1: plan all trees, find the common padded slot count
    plans = []
    maxM = 0
    nr = np.arange(1, N)
    for t in range(T):
        ev_enter, size, maxd = _tree_plan(par2[t])
        if maxd >= 4096:
            # reference's K=12 pointer doubling truncates paths longer than
            # 4096; the Euler scan computes the untruncated sum -> not
            # equivalent. Caller must use the exact fallback.
            return None, None, None
        ev_exit = ev_enter + 2 * size - 1
        pr = par2[t]
        nch = np.bincount(pr[1:], minlength=N)
        leaf = nch == 0
        keep = np.ones(twoN, bool)
        keep[ev_exit[leaf]] = False    # drop leaf exits
        keep[twoN - 1] = False         # drop trailing root pad
        newpos = (np.cumsum(keep) - 1).astype(np.int64)
        M = int(newpos[-1] + 1)
        maxM = max(maxM, M)
        plans.append((ev_enter, ev_exit, leaf, newpos))
    F = -(-maxM // (8 * P)) * 8        # slots per partition, padded to 8

    MP = P * F
    evA = np.zeros((T, MP), np.float32)
    evL = np.zeros((T, 2 * MP), LEVEL_DTYPE)   # per row: [lv | plv]
    q = np.empty((T, pix2.shape[1]), np.int32)
    for t in range(T):
        ev_enter, ev_exit, leaf, newpos = plans[t]
        at, lv, pr = attr2[t], level2[t], par2[t]
        en2 = newpos[ev_enter]
        ex2 = newpos[ev_exit]
        plv = lv[pr[nr]]
        ni = nr[~leaf[1:]]             # internal non-root nodes
        nl = nr[leaf[1:]]              # leaf nodes
        evA[t, en2[ni]] = at[ni]
        evA[t, ex2[ni]] = at[ni]
        afl = at[nl].copy()
        afl.view(np.uint32)[:] ^= 0x80000000   # sign-bit flip (integer op)
        evA[t, en2[nl]] = afl
        el = evL[t, :MP]
        ep = evL[t, MP:]
        el[en2[nr]] = lv[nr]
        ep[en2[nr]] = plv
        el[ex2[ni]] = plv[~leaf[1:]]   # swapped operands => exact negation
        ep[ex2[ni]] = lv[ni]
        q[t] = en2[np.clip(pix2[t], 0, N - 1)].astype(np.int32)

    thr_f = np.float32(thr.reshape(-1)[0])
    negthr = np.array(thr_f, np.float32)
    negthr.view(np.uint32)[...] ^= 0x80000000   # sign-bit flip (integer op)
    tri = np.triu(np.ones((P, P), np.float32), 1)   # tri[k,m]=1 iff k<m
    in_maps = []
    for c in range(N_CORES):
        tt = slice(c * TREES_PER_CORE, (c + 1) * TREES_PER_CORE)
        params = np.zeros((P, 4 * TREES_PER_CORE), np.float32)
        for k in range(TREES_PER_CORE):
            params[:, 4 * k] = thr_f
            params[:, 4 * k + 1] = negthr
            params[:, 4 * k + 2] = level2[c * TREES_PER_CORE + k, 0]
        in_maps.append({
            "evA": evA[tt].reshape(TREES_PER_CORE * P, F),
            "evL": evL[tt].reshape(TREES_PER_CORE, 2, P, F)
                .transpose(0, 2, 1, 3).reshape(TREES_PER_CORE * P, 2 * F),
            "params": params, "tri": tri})
    return in_maps, q, F


# ----------------------------------------------------------------------------
# Device program
# ----------------------------------------------------------------------------

def _build_nc(F, repeat=1, cfg=None):
    import concourse.bacc as bacc
    import concourse.mybir as mybir
    import concourse.tile as tile
    import concourse.bass as bass

    cfg = cfg or {}
    bufs = cfg.get("bufs", 2)
    version = cfg.get("version", "v3")   # "v3" | "v4"
    w1_eng = cfg.get("w1_eng", "vector")
    gneg_eng = cfg.get("gneg_eng", "act")  # v4: "act" | "dve"
    scan_bypass = cfg.get("scan_bypass", True)
    o1_via_act = cfg.get("o1_via_act", True)
    carry_in_scan = cfg.get("carry_in_scan", True)
    finadd_pool_stripe = cfg.get("finadd_pool_stripe", 1)  # trees on Pool
    o1_pool_stripe = cfg.get("o1_pool_stripe", 0)
    pipelined_store = cfg.get("pipelined_store", True)
    in_bufs = cfg.get("in_bufs", None)
    noscan = cfg.get("noscan", False)
    odma = cfg.get("odma", "scalar")     # "scalar" | "gpsimd" | "sync"
    mode = cfg.get("mode", "full")       # "full" | "dmaonly"

    f32 = mybir.dt.float32
    f16 = mybir.dt.float16
    op = mybir.AluOpType
    TP = TREES_PER_CORE * P

    nc = bacc.Bacc("TRN2", target_bir_lowering=False, debug=False,
                   num_devices=N_CORES)
    evA = nc.dram_tensor("evA", [TP, F], f32, kind="ExternalInput")
    lvdt = {np.dtype(np.uint8): mybir.dt.uint8,
            np.dtype(np.float16): f16}[np.dtype(LEVEL_DTYPE)]
    evL = nc.dram_tensor("evL", [TP, 2 * F], lvdt, kind="ExternalInput")
    params = nc.dram_tensor("params", [P, 4 * TREES_PER_CORE], f32,
                            kind="ExternalInput")
    triT = nc.dram_tensor("tri", [P, P], f32, kind="ExternalInput")
    Rout = nc.dram_tensor("R", [TP, F], f16, kind="ExternalOutput")

    with tile.TileContext(nc) as tc:
        with tc.tile_pool(name="sbuf", bufs=bufs) as pool, \
             tc.tile_pool(name="psum", space=bass.MemorySpace.PSUM,
                          bufs=2) as ppool:
            zero16 = pool.tile([P, 1], f16, tag="z16", bufs=1)
            nc.vector.memset(zero16[:], 0.0)
            prm = pool.tile([P, 4 * TREES_PER_CORE], f32, tag="prm", bufs=1)
            nc.sync.dma_start(prm, params.ap()[:, :])
            tri = pool.tile([P, P], f32, tag="tri", bufs=1)
            nc.sync.dma_start(tri, triT.ap()[:, :])
            odma_engs = {"scalar": [nc.scalar], "gpsimd": [nc.gpsimd],
                         "sync": [nc.sync],
                         "alt": [nc.scalar, nc.sync]}[odma]
            pending_store = None
            for i, t in enumerate([tt % TREES_PER_CORE for tt in
                                   range(TREES_PER_CORE * repeat)]):
                rows = slice(t * P, (t + 1) * P)
                ea = pool.tile([P, F], f32, tag="ea",
                               **({"bufs": in_bufs} if in_bufs else {}))
                nc.sync.dma_start(ea, evA.ap()[rows, :])
                el = pool.tile([P, 2 * F], lvdt, tag="el",
                               **({"bufs": in_bufs} if in_bufs else {}))
                nc.sync.dma_start(el, evL.ap()[rows, :])
                if mode == "dmaonly":
                    odma_engs[i % len(odma_engs)].dma_start(
                        Rout.ap()[rows, :], el[:, 0:F])
                    continue

                # Act first (leaf gate needs only ea), so on Act's in-order
                # queue this tree's Sign/Relu precede the previous tree's
                # (late-ready) result store
                if version == "v3" and o1_via_act:
                    sg = pool.tile([P, F], f16, tag="sg")
                    nc.scalar.activation(
                        sg[:], ea[:], mybir.ActivationFunctionType.Sign,
                        bias=prm[:, 4 * t + 1:4 * t + 2], scale=-1.0)
                    gneg = pool.tile([P, F], f16, tag="gneg")
                    nc.scalar.activation(
                        gneg[:], sg[:], mybir.ActivationFunctionType.Relu)
                if pipelined_store and pending_store is not None:
                    odma_engs[i % len(odma_engs)].dma_start(*pending_store)
                    pending_store = None

                # w1 = level - parent_level (exact fp16 negation pairs)
                w1_e = {"gpsimd": nc.gpsimd, "vector": nc.vector}[w1_eng]
                w1 = pool.tile([P, F], f16, tag="w1")
                w1_e.tensor_tensor(out=w1[:], in0=el[:, 0:F],
                                   in1=el[:, F:2 * F], op=op.subtract)
                # w2 = (attr >= thr) * w1 with fused per-partition row sums
                w2 = pool.tile([P, F], f16, tag="w2")
                rowsum = pool.tile([P, 1], f32, tag="rowsum")
                nc.vector.scalar_tensor_tensor(
                    out=w2[:], in0=ea[:], scalar=prm[:, 4 * t:4 * t + 1],
                    in1=w1[:], op0=op.is_ge, op1=op.mult,
                    accum_out=rowsum[:])

                # cross-partition exclusive prefix of rowsums on the (idle)
                # PE: excl[p] = sum_{k<p} rowsum[k] via strict-upper ones
                excl = ppool.tile([P, 1], f32, tag="excl")
                nc.tensor.matmul(excl[:], tri[:], rowsum[:])
                carry2 = pool.tile([P, 1], f32, tag="carry2")
                nc.vector.tensor_tensor(out=carry2[:], in0=excl[:],
                                        in1=prm[:, 4 * t + 2:4 * t + 3],
                                        op=op.add)

                ot = pool.tile([P, F], f16, tag="ot")
                if version == "v3":
                    o1 = pool.tile([P, F], f16, tag="o1")
                    rf = pool.tile([P, F], f16, tag="rf")
                    seed = carry2[:, 0:1] if carry_in_scan else 0.0
                    # scan before o1 on DVE's in-order queue: the scan does
                    # not depend on Act's gneg, so it must not sit behind o1
                    if noscan:
                        pass
                    elif scan_bypass:
                        nc.vector.tensor_tensor_scan(
                            out=rf[:], data0=w2[:], data1=w2[:],
                            initial=seed, op0=op.add, op1=op.bypass)
                    else:
                        nc.vector.tensor_tensor_scan(
                            out=rf[:], data0=w2[:],
                            data1=zero16[:].to_broadcast([P, F]),
                            initial=seed, op0=op.add, op1=op.add)
                    # o1 = (attr <= -thr)*w1: the leaf contributions
                    if o1_via_act:
                        o1_e = (nc.gpsimd
                                if t >= TREES_PER_CORE - int(o1_pool_stripe)
                                else nc.vector)
                        o1_e.tensor_tensor(out=o1[:], in0=gneg[:],
                                           in1=w1[:], op=op.mult)
                    else:
                        nc.vector.scalar_tensor_tensor(
                            out=o1[:], in0=ea[:],
                            scalar=prm[:, 4 * t + 1:4 * t + 2],
                            in1=w1[:], op0=op.is_le, op1=op.mult)
                    if noscan:
                        nc.vector.tensor_tensor(out=rf[:], in0=w2[:],
                                                in1=o1[:], op=op.add)
                    if carry_in_scan:
                        fin_e = (nc.gpsimd
                                 if t >= TREES_PER_CORE - int(finadd_pool_stripe)
                                 else nc.vector)
                        fin_e.tensor_tensor(out=ot[:], in0=rf[:],
                                            in1=o1[:], op=op.add)
                    else:
                        nc.vector.scalar_tensor_tensor(
                            out=ot[:], in0=rf[:], scalar=carry2[:, 0:1],
                            in1=o1[:], op0=op.add, op1=op.add)
                else:
                    # gneg = (attr <= -thr) as 0/1 (leaf slots carry
                    # sign-flipped attr, so Sign(-attr-thr) is +1 there)
                    gneg = pool.tile([P, F], f16, tag="gneg")
                    if gneg_eng == "act":
                        sg = pool.tile([P, F], f16, tag="sg")
                        nc.scalar.activation(
                            sg[:], ea[:], mybir.ActivationFunctionType.Sign,
                            bias=prm[:, 4 * t + 1:4 * t + 2], scale=-1.0)
                        nc.scalar.activation(
                            gneg[:], sg[:], mybir.ActivationFunctionType.Relu)
                    else:
                        nc.vector.tensor_scalar(
                            out=gneg[:], in0=ea[:],
                            scalar1=prm[:, 4 * t + 1:4 * t + 2],
                            scalar2=None, op0=op.is_le)
                    # o1 = gneg * w1: the leaf contributions
                    o1 = pool.tile([P, F], f16, tag="o1")
                    nc.gpsimd.tensor_tensor(out=o1[:], in0=gneg[:],
                                            in1=w1[:], op=op.mult)
                    # d0 = w2 - shift1(o1): with data1=o1 below, the scan
                    # emits inclscan(w2)[j] + o1[j] directly (telescoping).
                    # f32: the fp16-fp16 difference must stay exact or the
                    # telescoping drifts over the 393k-slot stream
                    d0 = pool.tile([P, F], f32, tag="d0")
                    nc.gpsimd.tensor_copy(d0[:, 0:1], w2[:, 0:1])
                    nc.gpsimd.tensor_tensor(out=d0[:, 1:F], in0=w2[:, 1:F],
                                            in1=o1[:, 0:F - 1],
                                            op=op.subtract)
                    # the scan IS the final output: fp32 state, fp16 out
                    if noscan:
                        nc.vector.tensor_tensor(out=ot[:], in0=w2[:],
                                                in1=o1[:], op=op.add)
                    else:
                        nc.vector.tensor_tensor_scan(
                            out=ot[:], data0=d0[:], data1=o1[:],
                            initial=carry2[:, 0:1], op0=op.add, op1=op.add)
                if pipelined_store:
                    pending_store = (Rout.ap()[rows, :], ot[:])
                else:
                    odma_engs[i % len(odma_engs)].dma_start(
                        Rout.ap()[rows, :], ot[:])
            if pending_store is not None:
                odma_engs[0].dma_start(*pending_store)
    nc.compile()
    return nc


def _get_nc(F):
    key = ("nc", F)
    if key not in _CACHE:
        _CACHE[key] = _build_nc(F)
    return _CACHE[key]


# ----------------------------------------------------------------------------
# Fallback: exact f32 emulation of the reference (invalid/cyclic trees only)
# ----------------------------------------------------------------------------

def _fallback_reference(attr, level, thr, parent, pixel_to_node):
    B, C, N = attr.shape
    # replicate reference's scaled-sigmoid gate semantics
    amin = attr.min(-1, keepdims=True)
    amax = attr.max(-1, keepdims=True)
    denom = np.maximum(amax - amin, np.float32(1e-6))
    a_s = ((attr - amin) / denom).astype(np.float32)
    t_n = ((np.float32(thr.reshape(-1)[0]) - amin) / denom).astype(np.float32)
    d = (a_s - t_n).astype(np.float32)
    soft = (1.0 / (1.0 + np.exp(-d.astype(np.float64)))).astype(np.float32)
    gate = (soft >= 0.5).astype(np.float32)
    pixel_to_node = np.clip(pixel_to_node, 0, N - 1)
    pl = np.take_along_axis(level, np.clip(parent, 0, N - 1).astype(np.int64),
                            axis=-1)
    s = gate * (level - pl)
    s[..., 0] = level[..., 0]
    s = np.concatenate([s, np.zeros((B, C, 1), np.float32)], axis=-1)
    p = np.concatenate([np.clip(parent, 0, N).astype(np.int32),
                        np.full((B, C, 1), N, np.int32)], axis=-1)
    p[..., 0] = N
    S = s.astype(np.float32)
    pp = p.astype(np.int64)
    for _ in range(12):
        S = (S + np.take_along_axis(S, pp, axis=-1)).astype(np.float32)
        pp = np.take_along_axis(pp, pp, axis=-1)
    S = S[..., :N]
    out = np.take_along_axis(S, pixel_to_node.astype(np.int64), axis=-1)
    HW = pixel_to_node.shape[-1]
    H = int(np.sqrt(HW))
    return out.reshape(B, C, H, HW // H).astype(np.float32)


# ----------------------------------------------------------------------------
# Entry point
# ----------------------------------------------------------------------------

def kernel(attr, level, thr_raw, parent, pixel_to_node):
    attr = np.asarray(attr, np.float32)
    level = np.asarray(level, np.float32)
    thr_raw = np.asarray(thr_raw, np.float32)
    parent = np.asarray(parent)
    pixel_to_node = np.asarray(pixel_to_node)
    B, C, N = attr.shape
    HW = pixel_to_node.shape[-1]
    H = int(np.sqrt(HW))

    par2 = parent.reshape(-1, N)
    valid = bool(np.all(par2[:, 1:] < np.arange(1, N)) and np.all(par2 >= 0)
                 and float(thr_raw.reshape(-1)[0]) > 0.0)
    if not valid or B * C != N_CORES * TREES_PER_CORE:
        return _fallback_reference(attr, level, thr_raw, parent, pixel_to_node)

    in_maps, q, F = _host_preprocess(attr, level, thr_raw, parent,
                                     pixel_to_node)
    if in_maps is None:  # depth >= 4096: doubling truncation applies
        return _fallback_reference(attr, level, thr_raw, parent,
                                   pixel_to_node)
    try:
        nc = _get_nc(F)
        from concourse.bass_utils import run_bass_kernel_spmd
        res = run_bass_kernel_spmd(nc, in_maps, core_ids=list(range(N_CORES)))
    except Exception as e:  # infra failure: still return a correct result
        import traceback
        traceback.print_exc()
        print(f"kernel: device path failed ({type(e).__name__}); "
              "falling back to host emulation")
        return _fallback_reference(attr, level, thr_raw, parent,
                                   pixel_to_node)

    out = np.empty((B * C, HW), np.float32)
    for c in range(N_CORES):
        R = res.results[c]["R"].astype(np.float32).reshape(TREES_PER_CORE,
                                                           P * F)
        for k in range(TREES_PER_CORE):
            t = c * TREES_PER_CORE + k
            out[t] = R[k][q[t]]
    return out.reshape(B, C, H, HW // H)


# revision 36
# speedup vs baseline: 2.6991x; 1.0129x over previous
"""Trainium2 kernel for nn_ConnectedThresholdLayer (gated connected-filter on
morphological max-trees + pixel reconstruction).

Mathematical reformulation (exactly equivalent to the reference on valid
trees, which setup_inputs always produces):

  The reference computes, per (b,c) tree, S[n] = sum of s[k] over the
  root->n path (pointer-doubling with K=12 covers depth < 4096; actual
  random-recursive-tree depth is ~35), with
      s[k] = gate[k] * (level[k] - level[parent[k]]),  s[root] = level[root]
      gate[k] = (sigmoid(a_scaled - thr_norm) >= 0.5)  ==  (attr[k] >= thr)
  (min-max scaling is strictly monotone, so the 0.5-sigmoid threshold
  reduces exactly to the raw comparison), then out[pix] = S[node[pix]].

  Path sums over a tree are an Euler-tour prefix scan: entering node k adds
  s[k], leaving subtracts it; the running sum at k's entry event equals
  S[k].  Leaf exit events are elided (the stream shrinks 2N -> ~1.5N): a
  leaf's entry slot carries its attr with the SIGN BIT flipped, so the scan
  gate (attr >= thr) reads 0 there (no carry pollution) while a second gate
  (attr <= -thr) recovers the leaf's own contribution in a post-scan add:

      out[j] = inclusive_scan(w2)[j] + (cross_partition_carry + root_level)
               + (attr[j] <= -thr) * w1[j]
      w2[j]  = (attr[j] >= thr) * w1[j],   w1[j] = lv[j] - plv[j]

  The host derives the (data-independent) tour layout from the int32
  `parent` tensor alone; the device does all floating-point math: gates,
  residues, the ~393k-element prefix scan per tree (per-partition scan +
  PE-matmul cross-partition carry), fully dense — no data-dependent
  addressing on device.

Precision: level payloads travel as fp16 (entry/exit contribution pairs are
exact fp16 negations — swapped operands — so path-sum error grows only with
tree depth ~35, not stream length).  attr stays fp32: the gate compare must
not flip near the threshold.  The scan state is fp32 in hardware regardless
of operand dtype; only the stored output rounds to fp16.

Engine placement (HW-tuned): DVE runs w1/w2/the carry-seeded scan and two
of the three final adds; one tree's final add is striped onto GPSIMD (slow,
but idle and off the critical chain); the leaf gate (Sign+Relu of
sign-flipped attr) runs on the otherwise-idle Activation engine; the
cross-partition carry is a strict-upper-triangular-ones matmul on the
otherwise-idle PE (its result seeds the scan's initial value).
Loads issue from SP and the result store from Activation so the transfers
overlap engine-side overheads (the DMA bus itself serializes ~370GB/s).

Sharding: trees are independent per (b,c); the 24 trees go 3-per-NeuronCore
across 8 cores (data parallel, zero cross-device communication).

Host does ONLY integer index planning (from `parent` / `pixel_to_node`) and
data marshaling (reordering input copies into event order, sign-bit flips
on the uint32 view, inverse map on the returned scan); every floating-point
operation on attr/level/thr values runs on the NeuronCores.
"""

import numpy as np

P = 128            # SBUF partitions
TREES_PER_CORE = 3
N_CORES = 8
LEVEL_DTYPE = np.float16  # u8 halves DMA but the DVE u8 path is slower on HW

_CACHE = {}


# ----------------------------------------------------------------------------
# Host-side integer planning (uses only `parent` / `pixel_to_node`)
# ----------------------------------------------------------------------------

def _tree_plan(parent):
    """parent: (N,) int with parent[n] < n for n >= 1.

    Returns ev_enter (N,) int64: position of each node's entry event in the
    2N-long Euler event stream.  Root (node 0) is excluded from the stream;
    positions 0 and 2N-1 are zero-contribution pads, and ev_enter[0] = 0
    (the running sum there is 0; the root's base level is added globally).
    """
    N = parent.shape[0]
    par = parent.astype(np.int64)
    ar = np.arange(N)

    # depth (= #edges to root) via pointer doubling with absorbing root
    val = (ar != 0).astype(np.int64)
    a = par.copy()
    a[0] = 0
    for _ in range(20):
        if not a.any():
            break
        val = val + val[a]
        a = a[a]
    depth = val
    maxd = int(depth.max())
    if maxd >= 4096:
        return None, None, maxd

    # subtree sizes, bottom-up by depth level
    size = np.ones(N, np.int64)
    order = np.argsort(depth, kind="stable")
    bounds = np.searchsorted(depth[order], np.arange(maxd + 2))
    for d in range(maxd, 0, -1):
        nodes = order[bounds[d]:bounds[d + 1]]
        if len(nodes) == 0:
            continue
        size += np.bincount(par[nodes], weights=size[nodes],
                            minlength=N).astype(np.int64)

    # prefix of earlier-sibling subtree sizes (children visited in index order)
    sibord = np.argsort(par[1:], kind="stable") + 1
    sz = size[sibord]
    cs = np.cumsum(sz) - sz
    pgroup = par[sibord]
    first = np.ones(len(sibord), bool)
    first[1:] = pgroup[1:] != pgroup[:-1]
    base = np.where(first, cs, 0)
    np.maximum.accumulate(base, out=base)
    bss = np.zeros(N, np.int64)
    bss[sibord] = cs - base

    # preorder index = path-sum of (1 + bss) excluding root, via doubling
    c = 1 + bss
    c[0] = 0
    S = c
    a = par.copy()
    a[0] = 0
    for _ in range(20):
        if not a.any():
            break
        S = S + S[a]
        a = a[a]
    pre = S
    ev_enter = 2 * pre - depth
    ev_enter[0] = 0
    return ev_enter, size, maxd


def _host_preprocess(attr, level, thr, parent, pixel_to_node):
    """Returns (in_maps for 8 cores, q (T, HW) int32 slot positions, F)."""
    B, C, N = attr.shape
    T = B * C
    twoN = 2 * N
    attr2 = np.ascontiguousarray(attr.reshape(T, N))
    level2 = np.ascontiguousarray(level.reshape(T, N))
    par2 = np.ascontiguousarray(parent.reshape(T, N))
    pix2 = pixel_to_node.reshape(T, -1)

    # pass 1: plan all trees, find the common padded slot count
    plans = []
    maxM = 0
    nr = np.arange(1, N)
    for t in range(T):
        ev_enter, size, maxd = _tree_plan(par2[t])
        if maxd >= 4096:
            # reference's K=12 pointer doubling truncates paths longer than
            # 4096; the Euler scan computes the untruncated sum -> not
            # equivalent. Caller must use the exact fallback.
            return None, None, None
        ev_exit = ev_enter + 2 * size - 1
        pr = par2[t]
        nch = np.bincount(pr[1:], minlength=N)
        leaf = nch == 0
        keep = np.ones(twoN, bool)
        keep[ev_exit[leaf]] = False    # drop leaf exits
        keep[twoN - 1] = False         # drop trailing root pad
        newpos = (np.cumsum(keep) - 1).astype(np.int64)
        M = int(newpos[-1] + 1)
        maxM = max(maxM, M)
        plans.append((ev_enter, ev_exit, leaf, newpos))
    F = -(-maxM // (8 * P)) * 8        # slots per partition, padded to 8

    MP = P * F
    evA = np.zeros((T, MP), np.float32)
    evL = np.zeros((T, 2 * MP), LEVEL_DTYPE)   # per row: [lv | plv]
    q = np.empty((T, pix2.shape[1]), np.int32)
    for t in range(T):
        ev_enter, ev_exit, leaf, newpos = plans[t]
        at, lv, pr = attr2[t], level2[t], par2[t]
        en2 = newpos[ev_enter]
        ex2 = newpos[ev_exit]
        plv = lv[pr[nr]]
        ni = nr[~leaf[1:]]             # internal non-root nodes
        nl = nr[leaf[1:]]              # leaf nodes
        evA[t, en2[ni]] = at[ni]
        evA[t, ex2[ni]] = at[ni]
        afl = at[nl].copy()
        afl.view(np.uint32)[:] ^= 0x80000000   # sign-bit flip (integer op)
        evA[t, en2[nl]] = afl
        el = evL[t, :MP]
        ep = evL[t, MP:]
        el[en2[nr]] = lv[nr]
        ep[en2[nr]] = plv
        el[ex2[ni]] = plv[~leaf[1:]]   # swapped operands => exact negation
        ep[ex2[ni]] = lv[ni]
        q[t] = en2[np.clip(pix2[t], 0, N - 1)].astype(np.int32)

    thr_f = np.float32(thr.reshape(-1)[0])
    negthr = np.array(thr_f, np.float32)
    negthr.view(np.uint32)[...] ^= 0x80000000   # sign-bit flip (integer op)
    tri = np.triu(np.ones((P, P), np.float32), 1)   # tri[k,m]=1 iff k<m
    in_maps = []
    for c in range(N_CORES):
        tt = slice(c * TREES_PER_CORE, (c + 1) * TREES_PER_CORE)
        params = np.zeros((P, 4 * TREES_PER_CORE), np.float32)
        for k in range(TREES_PER_CORE):
            params[:, 4 * k] = thr_f
            params[:, 4 * k + 1] = negthr
            params[:, 4 * k + 2] = level2[c * TREES_PER_CORE + k, 0]
        in_maps.append({
            "evA": evA[tt].reshape(TREES_PER_CORE * P, F),
            "evL": evL[tt].reshape(TREES_PER_CORE, 2, P, F)
                .transpose(0, 2, 1, 3).reshape(TREES_PER_CORE * P, 2 * F),
            "params": params, "tri": tri})
    return in_maps, q, F


# ----------------------------------------------------------------------------
# Device program
# ----------------------------------------------------------------------------

def _build_nc(F, repeat=1, cfg=None):
    import concourse.bacc as bacc
    import concourse.mybir as mybir
    import concourse.tile as tile
    import concourse.bass as bass

    cfg = cfg or {}
    bufs = cfg.get("bufs", 2)
    version = cfg.get("version", "v3")   # "v3" | "v4"
    w1_eng = cfg.get("w1_eng", "vector")
    gneg_eng = cfg.get("gneg_eng", "act")  # v4: "act" | "dve"
    scan_bypass = cfg.get("scan_bypass", True)
    o1_via_act = cfg.get("o1_via_act", True)
    carry_in_scan = cfg.get("carry_in_scan", True)
    finadd_pool_stripe = cfg.get("finadd_pool_stripe", 1)  # trees on Pool
    o1_pool_stripe = cfg.get("o1_pool_stripe", 0)
    pipelined_store = cfg.get("pipelined_store", True)
    el_first = cfg.get("el_first", False)
    in_bufs = cfg.get("in_bufs", None)
    noscan = cfg.get("noscan", False)
    odma = cfg.get("odma", "scalar")     # "scalar" | "gpsimd" | "sync"
    mode = cfg.get("mode", "full")       # "full" | "dmaonly"

    f32 = mybir.dt.float32
    f16 = mybir.dt.float16
    op = mybir.AluOpType
    TP = TREES_PER_CORE * P

    nc = bacc.Bacc("TRN2", target_bir_lowering=False, debug=False,
                   num_devices=N_CORES)
    evA = nc.dram_tensor("evA", [TP, F], f32, kind="ExternalInput")
    lvdt = {np.dtype(np.uint8): mybir.dt.uint8,
            np.dtype(np.float16): f16}[np.dtype(LEVEL_DTYPE)]
    evL = nc.dram_tensor("evL", [TP, 2 * F], lvdt, kind="ExternalInput")
    params = nc.dram_tensor("params", [P, 4 * TREES_PER_CORE], f32,
                            kind="ExternalInput")
    triT = nc.dram_tensor("tri", [P, P], f32, kind="ExternalInput")
    Rout = nc.dram_tensor("R", [TP, F], f16, kind="ExternalOutput")

    with tile.TileContext(nc) as tc:
        with tc.tile_pool(name="sbuf", bufs=bufs) as pool, \
             tc.tile_pool(name="psum", space=bass.MemorySpace.PSUM,
                          bufs=2) as ppool:
            zero16 = pool.tile([P, 1], f16, tag="z16", bufs=1)
            nc.vector.memset(zero16[:], 0.0)
            prm = pool.tile([P, 4 * TREES_PER_CORE], f32, tag="prm", bufs=1)
            nc.sync.dma_start(prm, params.ap()[:, :])
            tri = pool.tile([P, P], f32, tag="tri", bufs=1)
            nc.sync.dma_start(tri, triT.ap()[:, :])
            odma_engs = {"scalar": [nc.scalar], "gpsimd": [nc.gpsimd],
                         "sync": [nc.sync],
                         "alt": [nc.scalar, nc.sync]}[odma]
            pending_store = None
            for i, t in enumerate([tt % TREES_PER_CORE for tt in
                                   range(TREES_PER_CORE * repeat)]):
                rows = slice(t * P, (t + 1) * P)
                ea = pool.tile([P, F], f32, tag="ea",
                               **({"bufs": in_bufs} if in_bufs else {}))
                el = pool.tile([P, 2 * F], lvdt, tag="el",
                               **({"bufs": in_bufs} if in_bufs else {}))
                if el_first:
                    # w1 (head of the DVE chain) needs only el; ea is
                    # consumed later (w2 / Act Sign) — load el first
                    nc.sync.dma_start(el, evL.ap()[rows, :])
                    nc.sync.dma_start(ea, evA.ap()[rows, :])
                else:
                    nc.sync.dma_start(ea, evA.ap()[rows, :])
                    nc.sync.dma_start(el, evL.ap()[rows, :])
                if mode == "dmaonly":
                    odma_engs[i % len(odma_engs)].dma_start(
                        Rout.ap()[rows, :], el[:, 0:F])
                    continue

                # Act first (leaf gate needs only ea), so on Act's in-order
                # queue this tree's Sign/Relu precede the previous tree's
                # (late-ready) result store
                if version == "v3" and o1_via_act:
                    sg = pool.tile([P, F], f16, tag="sg")
                    nc.scalar.activation(
                        sg[:], ea[:], mybir.ActivationFunctionType.Sign,
                        bias=prm[:, 4 * t + 1:4 * t + 2], scale=-1.0)
                    gneg = pool.tile([P, F], f16, tag="gneg")
                    nc.scalar.activation(
                        gneg[:], sg[:], mybir.ActivationFunctionType.Relu)
                if pipelined_store and pending_store is not None:
                    odma_engs[i % len(odma_engs)].dma_start(*pending_store)
                    pending_store = None

                # w1 = level - parent_level (exact fp16 negation pairs)
                w1_e = {"gpsimd": nc.gpsimd, "vector": nc.vector}[w1_eng]
                w1 = pool.tile([P, F], f16, tag="w1")
                w1_e.tensor_tensor(out=w1[:], in0=el[:, 0:F],
                                   in1=el[:, F:2 * F], op=op.subtract)
                # w2 = (attr >= thr) * w1 with fused per-partition row sums
                w2 = pool.tile([P, F], f16, tag="w2")
                rowsum = pool.tile([P, 1], f32, tag="rowsum")
                nc.vector.scalar_tensor_tensor(
                    out=w2[:], in0=ea[:], scalar=prm[:, 4 * t:4 * t + 1],
                    in1=w1[:], op0=op.is_ge, op1=op.mult,
                    accum_out=rowsum[:])

                # cross-partition exclusive prefix of rowsums on the (idle)
                # PE: excl[p] = sum_{k<p} rowsum[k] via strict-upper ones
                excl = ppool.tile([P, 1], f32, tag="excl")
                nc.tensor.matmul(excl[:], tri[:], rowsum[:])
                carry2 = pool.tile([P, 1], f32, tag="carry2")
                nc.vector.tensor_tensor(out=carry2[:], in0=excl[:],
                                        in1=prm[:, 4 * t + 2:4 * t + 3],
                                        op=op.add)

                ot = pool.tile([P, F], f16, tag="ot")
                if version == "v3":
                    o1 = pool.tile([P, F], f16, tag="o1")
                    rf = pool.tile([P, F], f16, tag="rf")
                    seed = carry2[:, 0:1] if carry_in_scan else 0.0
                    # scan before o1 on DVE's in-order queue: the scan does
                    # not depend on Act's gneg, so it must not sit behind o1
                    if noscan:
                        pass
                    elif scan_bypass:
                        nc.vector.tensor_tensor_scan(
                            out=rf[:], data0=w2[:], data1=w2[:],
                            initial=seed, op0=op.add, op1=op.bypass)
                    else:
                        nc.vector.tensor_tensor_scan(
                            out=rf[:], data0=w2[:],
                            data1=zero16[:].to_broadcast([P, F]),
                            initial=seed, op0=op.add, op1=op.add)
                    # o1 = (attr <= -thr)*w1: the leaf contributions
                    if o1_via_act:
                        o1_e = (nc.gpsimd
                                if t >= TREES_PER_CORE - int(o1_pool_stripe)
                                else nc.vector)
                        o1_e.tensor_tensor(out=o1[:], in0=gneg[:],
                                           in1=w1[:], op=op.mult)
                    else:
                        nc.vector.scalar_tensor_tensor(
                            out=o1[:], in0=ea[:],
                            scalar=prm[:, 4 * t + 1:4 * t + 2],
                            in1=w1[:], op0=op.is_le, op1=op.mult)
                    if noscan:
                        nc.vector.tensor_tensor(out=rf[:], in0=w2[:],
                                                in1=o1[:], op=op.add)
                    if carry_in_scan:
                        fin_e = (nc.gpsimd
                                 if t >= TREES_PER_CORE - int(finadd_pool_stripe)
                                 else nc.vector)
                        fin_e.tensor_tensor(out=ot[:], in0=rf[:],
                                            in1=o1[:], op=op.add)
                    else:
                        nc.vector.scalar_tensor_tensor(
                            out=ot[:], in0=rf[:], scalar=carry2[:, 0:1],
                            in1=o1[:], op0=op.add, op1=op.add)
                else:
                    # gneg = (attr <= -thr) as 0/1 (leaf slots carry
                    # sign-flipped attr, so Sign(-attr-thr) is +1 there)
                    gneg = pool.tile([P, F], f16, tag="gneg")
                    if gneg_eng == "act":
                        sg = pool.tile([P, F], f16, tag="sg")
                        nc.scalar.activation(
                            sg[:], ea[:], mybir.ActivationFunctionType.Sign,
                            bias=prm[:, 4 * t + 1:4 * t + 2], scale=-1.0)
                        nc.scalar.activation(
                            gneg[:], sg[:], mybir.ActivationFunctionType.Relu)
                    else:
                        nc.vector.tensor_scalar(
                            out=gneg[:], in0=ea[:],
                            scalar1=prm[:, 4 * t + 1:4 * t + 2],
                            scalar2=None, op0=op.is_le)
                    # o1 = gneg * w1: the leaf contributions
                    o1 = pool.tile([P, F], f16, tag="o1")
                    nc.gpsimd.tensor_tensor(out=o1[:], in0=gneg[:],
                                            in1=w1[:], op=op.mult)
                    # d0 = w2 - shift1(o1): with data1=o1 below, the scan
                    # emits inclscan(w2)[j] + o1[j] directly (telescoping).
                    # f32: the fp16-fp16 difference must stay exact or the
                    # telescoping drifts over the 393k-slot stream
                    d0 = pool.tile([P, F], f32, tag="d0")
                    nc.gpsimd.tensor_copy(d0[:, 0:1], w2[:, 0:1])
                    nc.gpsimd.tensor_tensor(out=d0[:, 1:F], in0=w2[:, 1:F],
                                            in1=o1[:, 0:F - 1],
                                            op=op.subtract)
                    # the scan IS the final output: fp32 state, fp16 out
                    if noscan:
                        nc.vector.tensor_tensor(out=ot[:], in0=w2[:],
                                                in1=o1[:], op=op.add)
                    else:
                        nc.vector.tensor_tensor_scan(
                            out=ot[:], data0=d0[:], data1=o1[:],
                            initial=carry2[:, 0:1], op0=op.add, op1=op.add)
                if pipelined_store:
                    pending_store = (Rout.ap()[rows, :], ot[:])
                else:
                    odma_engs[i % len(odma_engs)].dma_start(
                        Rout.ap()[rows, :], ot[:])
            if pending_store is not None:
                odma_engs[0].dma_start(*pending_store)
    nc.compile()
    return nc


def _get_nc(F):
    key = ("nc", F)
    if key not in _CACHE:
        _CACHE[key] = _build_nc(F)
    return _CACHE[key]


# ----------------------------------------------------------------------------
# Fallback: exact f32 emulation of the reference (invalid/cyclic trees only)
# ----------------------------------------------------------------------------

def _fallback_reference(attr, level, thr, parent, pixel_to_node):
    B, C, N = attr.shape
    # replicate reference's scaled-sigmoid gate semantics
    amin = attr.min(-1, keepdims=True)
    amax = attr.max(-1, keepdims=True)
    denom = np.maximum(amax - amin, np.float32(1e-6))
    a_s = ((attr - amin) / denom).astype(np.float32)
    t_n = ((np.float32(thr.reshape(-1)[0]) - amin) / denom).astype(np.float32)
    d = (a_s - t_n).astype(np.float32)
    soft = (1.0 / (1.0 + np.exp(-d.astype(np.float64)))).astype(np.float32)
    gate = (soft >= 0.5).astype(np.float32)
    pixel_to_node = np.clip(pixel_to_node, 0, N - 1)
    pl = np.take_along_axis(level, np.clip(parent, 0, N - 1).astype(np.int64),
                            axis=-1)
    s = gate * (level - pl)
    s[..., 0] = level[..., 0]
    s = np.concatenate([s, np.zeros((B, C, 1), np.float32)], axis=-1)
    p = np.concatenate([np.clip(parent, 0, N).astype(np.int32),
                        np.full((B, C, 1), N, np.int32)], axis=-1)
    p[..., 0] = N
    S = s.astype(np.float32)
    pp = p.astype(np.int64)
    for _ in range(12):
        S = (S + np.take_along_axis(S, pp, axis=-1)).astype(np.float32)
        pp = np.take_along_axis(pp, pp, axis=-1)
    S = S[..., :N]
    out = np.take_along_axis(S, pixel_to_node.astype(np.int64), axis=-1)
    HW = pixel_to_node.shape[-1]
    H = int(np.sqrt(HW))
    return out.reshape(B, C, H, HW // H).astype(np.float32)


# ----------------------------------------------------------------------------
# Entry point
# ----------------------------------------------------------------------------

def kernel(attr, level, thr_raw, parent, pixel_to_node):
    attr = np.asarray(attr, np.float32)
    level = np.asarray(level, np.float32)
    thr_raw = np.asarray(thr_raw, np.float32)
    parent = np.asarray(parent)
    pixel_to_node = np.asarray(pixel_to_node)
    B, C, N = attr.shape
    HW = pixel_to_node.shape[-1]
    H = int(np.sqrt(HW))

    par2 = parent.reshape(-1, N)
    valid = bool(np.all(par2[:, 1:] < np.arange(1, N)) and np.all(par2 >= 0)
                 and float(thr_raw.reshape(-1)[0]) > 0.0)
    if not valid or B * C != N_CORES * TREES_PER_CORE:
        return _fallback_reference(attr, level, thr_raw, parent, pixel_to_node)

    in_maps, q, F = _host_preprocess(attr, level, thr_raw, parent,
                                     pixel_to_node)
    if in_maps is None:  # depth >= 4096: doubling truncation applies
        return _fallback_reference(attr, level, thr_raw, parent,
                                   pixel_to_node)
    try:
        nc = _get_nc(F)
        from concourse.bass_utils import run_bass_kernel_spmd
        res = run_bass_kernel_spmd(nc, in_maps, core_ids=list(range(N_CORES)))
    except Exception as e:  # infra failure: still return a correct result
        import traceback
        traceback.print_exc()
        print(f"kernel: device path failed ({type(e).__name__}); "
              "falling back to host emulation")
        return _fallback_reference(attr, level, thr_raw, parent,
                                   pixel_to_node)

    out = np.empty((B * C, HW), np.float32)
    for c in range(N_CORES):
        R = res.results[c]["R"].astype(np.float32).reshape(TREES_PER_CORE,
                                                           P * F)
        for k in range(TREES_PER_CORE):
            t = c * TREES_PER_CORE + k
            out[t] = R[k][q[t]]
    return out.reshape(B, C, H, HW // H)
